# revision 5
# baseline (speedup 1.0000x reference)
"""GPT-2 (L=12, E=1024, H=16, T=1024, B=8) forward on 8 Trainium2 NeuronCores.

Data-parallel over batch (1 sequence per core) for the 12 transformer layers;
vocab-parallel lm_head (each core computes a V/8 logits shard for all 8
sequences) as a second tiny NEFF, with the 8 last-position hidden vectors
gathered on host between the phases.

v3 (vs baseline f32r kernel):
  - all matmuls in bf16 (same PE rate as f32r but: half the DMA bytes,
    ldweights at 1 cycle/row, no 4x penalty on <256-wide moving chunks,
    transposes at 1 cycle/row)
  - softmax denominator folded into the AV matmul as a 65th ones-column of
    the V stationary tile (eliminates the separate PE denominator matmul)
  - per-token-tile layernorm pipelined behind proj / fc2-last-slab (PE never
    drains at sublayer boundaries, keeping the HAM clock warm)
  - scores/exp of head h software-pipelined with AV of head h-1
    (double-buffered attT)
  - weights host-packed into per-DMA-tile contiguous layouts (1KB/partition
    lines instead of 256-512B strided)

Host-side preprocessing (all linear folds, no model compute):
  - embedding gather x0 = wte[idx] + wpe  (pure indexing)
  - layernorm scale folded into the following matmul weights
  - sqrt(1/sqrt(D)) folded into both W_q and W_k
  - wte transposed (+ lnf scale) in bf16 for the lm_head
"""

import os
import sys

import numpy as np

sys.path.insert(0, "/opt/trn_rl_repo")

V, BLK, L, H, E = 50257, 1024, 12, 16, 1024
T = 1024
D = E // H  # 64
NCORES = 8
E3 = 3 * E
E4 = 4 * E
NTT = T // 128  # 8 token tiles
NEO = E // 128  # 8 embed tiles
VSH = (V + NCORES - 1) // NCORES  # 6283 vocab shard
VSP = 13 * 512  # 6656 padded shard width
NL = int(os.environ.get("GPT_NL", str(L)))

_CACHE = {}


def _build_phase1(nl):
    import concourse.mybir as mybir
    import concourse.tile as tile
    from concourse import bacc
    from concourse.masks import make_identity

    f32 = mybir.dt.float32
    bf = mybir.dt.bfloat16
    u16 = mybir.dt.uint16
    AF = mybir.ActivationFunctionType
    ALU = mybir.AluOpType

    nc = bacc.Bacc("TRN2", target_bir_lowering=False)

    x0 = nc.dram_tensor("x0", [T, E], f32, kind="ExternalInput")
    # bf16 weights as uint16 carriers (bitcast at DMA), packed per-DMA-tile:
    # lhsT-style [.., eo, p, ct, m]; rhs-style [.., k, p, n]
    wqk = nc.dram_tensor("wqk", [nl, 4, NEO, 128, 4, 128], u16, kind="ExternalInput")
    wv = nc.dram_tensor("wv", [nl, 2, NEO, 128, 512], u16, kind="ExternalInput")
    wproj = nc.dram_tensor("wproj", [nl, 2, NEO, 128, 512], u16, kind="ExternalInput")
    wfc = nc.dram_tensor("wfc", [nl, 4, 2, NEO, 128, 4, 128], u16, kind="ExternalInput")
    wfc2 = nc.dram_tensor("wfc2", [nl, 4, 2, NEO, 128, 512], u16, kind="ExternalInput")
    xlast = nc.dram_tensor("xlast", [1, E], f32, kind="ExternalOutput")

    with tile.TileContext(nc) as tc:
        import contextlib

        ctx = contextlib.ExitStack()
        with ctx:
            singles = ctx.enter_context(tc.tile_pool(name="singles", bufs=1))
            wl = ctx.enter_context(tc.tile_pool(name="wl", bufs=20))  # [128,4,128] bf
            wr = ctx.enter_context(tc.tile_pool(name="wr", bufs=26))  # [128,512] bf
            hpool = ctx.enter_context(tc.tile_pool(name="hpool", bufs=2))
            stat = ctx.enter_context(tc.tile_pool(name="stat", bufs=2))
            bc = ctx.enter_context(tc.tile_pool(name="bc", bufs=2))
            ps = ctx.enter_context(tc.tile_pool(name="ps", bufs=6, space="PSUM"))
            psb = ctx.enter_context(tc.tile_pool(name="psb", bufs=1, space="PSUM"))

            # ---- persistent tiles ----
            X = singles.tile([128, NTT, T], f32)  # residual [t, tt, e]
            HT = singles.tile([128, NEO, T], bf)  # ln-out transposed [e, eo, t]
            AOT = singles.tile([128, NEO, T], bf)  # attn outT [c, co, t]
            scrA = ctx.enter_context(tc.tile_pool(name="scrA", bufs=1))
            scrB = ctx.enter_context(tc.tile_pool(name="scrB", bufs=1))

            identb = singles.tile([128, 128], bf)
            make_identity(nc, identb)
            # maskT[k, q] = 0 if q >= k else -30  (additive, transposed causal)
            maskT = singles.tile([128, 128], f32)
            nc.gpsimd.memset(maskT, 0.0)
            nc.gpsimd.affine_select(
                out=maskT,
                in_=maskT,
                compare_op=ALU.is_ge,
                fill=-30.0,
                base=0,
                pattern=[[1, 128]],
                channel_multiplier=-1,
            )
            eps_t = singles.tile([128, 1], f32)
            nc.gpsimd.memset(eps_t, 1e-5)
            f32r = mybir.dt.float32r
            ones_f = singles.tile([1, 64], f32)
            nc.gpsimd.memset(ones_f, 1.0)
            ones_r = singles.tile([1, 64], f32r)
            nc.scalar.activation(ones_r, ones_f, AF.Copy)

            # ---- load x0 ----
            x0v = x0[:, :].rearrange("(tt p) e -> p tt e", p=128)
            for tt in range(NTT):
                nc.sync.dma_start(X[:, tt, :], x0v[:, tt, :])

            def ln_tt(src, tt):
                """LN(src[t,tt,:]) -> HT[:, :, tt*128:+128] bf16 via PE transposes."""
                st = stat.tile([128, 2, 6], f32, tag="bnst", name=f"st{tt}")
                for c in range(2):
                    nc.vector.bn_stats(st[:, c, :], src[:, tt, c * 512 : (c + 1) * 512])
                mv = stat.tile([128, 2], f32, tag="bnmv", name=f"mv{tt}")
                nc.vector.bn_aggr(mv, st)
                rstd = stat.tile([128, 1], f32, tag="rstd", name=f"rs{tt}")
                nc.scalar.activation(rstd, mv[:, 1:2], AF.Sqrt, bias=eps_t)
                nc.vector.reciprocal(rstd, rstd)
                h = hpool.tile([128, T], bf, tag="h", name=f"h{tt}")
                nc.vector.tensor_scalar(
                    out=h,
                    in0=src[:, tt, :],
                    scalar1=mv[:, 0:1],
                    scalar2=rstd,
                    op0=ALU.subtract,
                    op1=ALU.mult,
                )
                for half in range(2):
                    ptr = ps.tile([128, 4, 128], bf, tag="mm", name=f"ptr{tt}_{half}")
                    for eq in range(4):
                        eo = half * 4 + eq
                        nc.tensor.transpose(ptr[:, eq, :], h[:, eo * 128 : (eo + 1) * 128], identb)
                    nc.scalar.activation(
                        HT[:, half * 4 : half * 4 + 4, tt * 128 : (tt + 1) * 128],
                        ptr,
                        AF.Copy,
                    )

            def mm_lhsw_to_ct(dst, wdram_l, n_ct, act=AF.Copy):
                """dst[:, ct, t] (bf16) = W[ct].T @ HT for ct in range(n_ct).
                wdram_l: [NEO, 128, 4, 128] packed quarter (n_ct=4) or
                list of two for n_ct=8."""
                quarters = wdram_l if isinstance(wdram_l, list) else [wdram_l]
                for qi, wq in enumerate(quarters):
                    wts = []
                    for eo in range(NEO):
                        wt = wl.tile([128, 4, 128], bf, tag="w_l", name=f"wl{qi}_{eo}")
                        nc.sync.dma_start(wt, wq[eo].bitcast(bf))
                        wts.append(wt)
                    for ct in range(4):
                        for ch in range(2):
                            pt = ps.tile([128, 512], f32, tag="mm", name=f"p{qi}_{ct}_{ch}")
                            for eo in range(NEO):
                                nc.tensor.matmul(
                                    pt,
                                    wts[eo][:, ct, :],
                                    HT[:, eo, ch * 512 : (ch + 1) * 512],
                                    start=(eo == 0),
                                    stop=(eo == NEO - 1),
                                )
                            nc.scalar.activation(
                                dst[:, qi * 4 + ct, ch * 512 : (ch + 1) * 512], pt, act
                            )

            def rhs_group(lhsT3, wts, tt, name):
                pt = ps.tile([128, 512], f32, tag="mm", name=name)
                for k in range(NEO):
                    nc.tensor.matmul(
                        pt,
                        lhsT3[:, k, tt * 128 : (tt + 1) * 128],
                        wts[k],
                        start=(k == 0),
                        stop=(k == NEO - 1),
                    )
                return pt

            def load_wr(wdram, n, name):
                wts = []
                for k in range(n):
                    wt = wr.tile([128, 512], bf, tag="w_r", name=f"{name}{k}")
                    nc.sync.dma_start(wt, wdram[k].bitcast(bf))
                    wts.append(wt)
                return wts

            for l in range(nl):
                # ===== attention =====
                if l == 0:
                    for tt in range(NTT):
                        ln_tt(X, tt)
                for g in range(2):  # head groups of 8 heads (512 c-cols)
                    qkg = scrA.tile([128, 8, T], bf, tag="scrA", name=f"qkg{l}_{g}")
                    vatt = scrB.tile([128, 20544], bf, tag="scrB", name=f"vatt{l}_{g}")
                    QG = qkg[:, 0:4, :]
                    KG = qkg[:, 4:8, :]
                    # V with a 65th ones-column per head (softmax denom trick)
                    VG = vatt[:, 0:4160].rearrange("p (j h c) -> p j h c", j=NTT, h=8)
                    # double-buffered non-ragged attT [parity, j, q]
                    attT = vatt[:, 4160:20544].rearrange("p (b j q) -> p b j q", b=2, j=NTT)
                    mm_lhsw_to_ct(QG, wqk[l, g], 4)
                    mm_lhsw_to_ct(KG, wqk[l, 2 + g], 4)

                    vw = load_wr(wv[l, g], NEO, f"vw{g}")
                    for tt in range(NTT):
                        pt = rhs_group(HT, vw, tt, f"pv{tt}")
                        nc.scalar.activation(VG[:, tt, :, 0:64], pt, AF.Copy)
                    nc.gpsimd.memset(VG[:, :, :, 64:65], 1.0)

                    def scores(hh):
                        pb = hh % 2
                        ct, ro = hh // 2, (hh % 2) * 64
                        qT = QG[ro : ro + 64, ct, :]  # [64, 1024] bf16
                        kT = KG[ro : ro + 64, ct, :]
                        for j in range(NTT):
                            q0 = j * 128
                            for ch in range(q0, T, 512):
                                w = min(512, T - ch)
                                pa = ps.tile([128, 512], f32, tag="mm", name=f"pa{j}_{ch}")
                                nc.tensor.matmul(
                                    pa[:, :w],
                                    kT[:, q0 : q0 + 128],
                                    qT[:, ch : ch + w],
                                    start=True,
                                    stop=True,
                                )
                                if ch == q0:  # causal mask on diagonal block
                                    nc.vector.tensor_tensor(pa[:, :128], pa[:, :128], maskT, ALU.add)
                                nc.scalar.activation(attT[:, pb, j, ch : ch + w], pa[:, :w], AF.Exp)

                    def av(hh):
                        pb = hh % 2
                        h_glob = g * 8 + hh
                        av_ps = psb.tile([65, 1024], f32, tag="av", name=f"av{l}_{h_glob}")
                        for j in range(NTT):
                            vsl = VG[:, j, hh, 0:65]
                            for ca in range(2):
                                s = max(ca * 512, j * 128)
                                if s >= (ca + 1) * 512:
                                    continue
                                w = (ca + 1) * 512 - s
                                nc.tensor.matmul(
                                    av_ps[:, s : s + w],
                                    vsl,
                                    attT[:, pb, j, s : s + w],
                                    start=(j == 0),
                                    stop=(j == (3 if ca == 0 else 7)),
                                    skip_group_check=True,
                                )
                        # denominator: evac row 64, PE-broadcast to 64 rows,
                        # reciprocal on 64 lanes, multiply (all on-chip)
                        dsb = stat.tile([1, 1024], f32r, tag="dsb", name=f"ds{l}_{h_glob}")
                        nc.scalar.activation(dsb, av_ps[64:65, :], AF.Copy)
                        den64 = bc.tile([64, 1024], f32, tag="rdb", name=f"dn{l}_{h_glob}")
                        for c2 in range(2):
                            pb2 = ps.tile([64, 512], f32, tag="mm", name=f"pb{l}_{h_glob}_{c2}")
                            nc.tensor.matmul(
                                pb2, ones_r, dsb[:, c2 * 512 : (c2 + 1) * 512],
                                start=True, stop=True,
                            )
                            nc.vector.tensor_copy(den64[:, c2 * 512 : (c2 + 1) * 512], pb2)
                        nc.vector.reciprocal(den64, den64)
                        co, ro2 = h_glob // 2, (h_glob % 2) * 64
                        nc.vector.tensor_tensor(
                            AOT[ro2 : ro2 + 64, co, :], av_ps[0:64, :], den64, ALU.mult
                        )

                    for hh in range(8):
                        scores(hh)
                        if hh > 0:
                            av(hh - 1)
                    av(7)

                # proj + residual, mlp-LN pipelined per token tile
                pw = load_wr(wproj[l, 0], NEO, "pw0") + load_wr(wproj[l, 1], NEO, "pw1")
                for tt in range(NTT):
                    for ch in range(2):
                        pt = rhs_group(AOT, pw[ch * NEO : (ch + 1) * NEO], tt, f"pp{tt}_{ch}")
                        nc.vector.tensor_tensor(
                            X[:, tt, ch * 512 : (ch + 1) * 512],
                            X[:, tt, ch * 512 : (ch + 1) * 512],
                            pt,
                            ALU.add,
                        )
                    ln_tt(X, tt)

                # ===== mlp =====
                FC2A = scrB.tile([128, NTT, T], f32, tag="scrB", name=f"fc2a{l}")
                for slab in range(4):  # 4E in 4 slabs of 1024
                    H1T = scrA.tile([128, 8, T], bf, tag="scrA", name=f"h1t{l}_{slab}")
                    mm_lhsw_to_ct(
                        H1T, [wfc[l, slab, 0], wfc[l, slab, 1]], 8, act=AF.Gelu_apprx_tanh
                    )
                    f2w = load_wr(wfc2[l, slab, 0], NEO, f"f2a{slab}") + load_wr(
                        wfc2[l, slab, 1], NEO, f"f2b{slab}"
                    )
                    last = slab == 3
                    for tt in range(NTT):
                        for ch in range(2):
                            pt = rhs_group(H1T, f2w[ch * NEO : (ch + 1) * NEO], tt, f"pf{slab}_{tt}_{ch}")
                            if slab == 0:
                                nc.vector.tensor_tensor(
                                    FC2A[:, tt, ch * 512 : (ch + 1) * 512],
                                    X[:, tt, ch * 512 : (ch + 1) * 512],
                                    pt,
                                    ALU.add,
                                )
                            elif not last:
                                nc.vector.tensor_tensor(
                                    FC2A[:, tt, ch * 512 : (ch + 1) * 512],
                                    FC2A[:, tt, ch * 512 : (ch + 1) * 512],
                                    pt,
                                    ALU.add,
                                )
                            else:
                                nc.vector.tensor_tensor(
                                    X[:, tt, ch * 512 : (ch + 1) * 512],
                                    FC2A[:, tt, ch * 512 : (ch + 1) * 512],
                                    pt,
                                    ALU.add,
                                )
                        if last and l + 1 < nl:
                            ln_tt(X, tt)

            # ===== final layernorm on last token tile, emit last row =====
            st = stat.tile([128, 2, 6], f32, tag="bnst", name="stf")
            for c in range(2):
                nc.vector.bn_stats(st[:, c, :], X[:, NTT - 1, c * 512 : (c + 1) * 512])
            mv = stat.tile([128, 2], f32, tag="bnmv", name="mvf")
            nc.vector.bn_aggr(mv, st)
            rstd = stat.tile([128, 1], f32, tag="rstd", name="rsf")
            nc.scalar.activation(rstd, mv[:, 1:2], AF.Sqrt, bias=eps_t)
            nc.vector.reciprocal(rstd, rstd)
            xn = hpool.tile([128, T], f32, tag="xn", name="xnf")
            nc.vector.tensor_scalar(
                out=xn,
                in0=X[:, NTT - 1, :],
                scalar1=mv[:, 0:1],
                scalar2=rstd,
                op0=ALU.subtract,
                op1=ALU.mult,
            )
            nc.sync.dma_start(xlast[:, :], xn[127:128, :])

    nc.compile()
    return nc


def _build_phase2():
    import concourse.mybir as mybir
    import concourse.tile as tile
    from concourse import bacc

    f32 = mybir.dt.float32
    bf = mybir.dt.bfloat16
    u16 = mybir.dt.uint16
    AF = mybir.ActivationFunctionType

    nc = bacc.Bacc("TRN2", target_bir_lowering=False)
    xallt = nc.dram_tensor("xallt", [E, NCORES], u16, kind="ExternalInput")
    wtet = nc.dram_tensor("wtet", [E, VSP], u16, kind="ExternalInput")
    lg = nc.dram_tensor("lg", [NCORES, VSP], f32, kind="ExternalOutput")

    with tile.TileContext(nc) as tc:
        with (
            tc.tile_pool(name="s", bufs=1) as s,
            tc.tile_pool(name="w", bufs=6) as w,
            tc.tile_pool(name="o", bufs=4) as o,
            tc.tile_pool(name="p", bufs=4, space="PSUM") as p,
        ):
            xt = s.tile([128, NEO, NCORES], bf)
            nc.sync.dma_start(xt, xallt[:, :].rearrange("(eo p) s -> p eo s", p=128).bitcast(bf))
            for vc in range(VSP // 512):
                pt = p.tile([NCORES, 512], f32, tag="p", name=f"p{vc}")
                for eo in range(NEO):
                    wt = w.tile([128, 512], bf, tag="w", name=f"w{vc}_{eo}")
                    nc.sync.dma_start(
                        wt, wtet[eo * 128 : (eo + 1) * 128, vc * 512 : (vc + 1) * 512].bitcast(bf)
                    )
                    nc.tensor.matmul(pt, xt[:, eo, :], wt, start=(eo == 0), stop=(eo == NEO - 1))
                ot = o.tile([NCORES, 512], f32, tag="o", name=f"o{vc}")
                nc.scalar.activation(ot, pt, AF.Copy)
                nc.sync.dma_start(lg[:, vc * 512 : (vc + 1) * 512], ot)
    nc.compile()
    return nc


def _host_prep(idx, wte, wpe, ln1_w, ln1_b, attn_w, attn_b, proj_w, proj_b,
               ln2_w, ln2_b, fc_w, fc_b, fc2_w, fc2_b, lnf_w, lnf_b, nl):
    import ml_dtypes

    bf = ml_dtypes.bfloat16
    f = np.float32
    idx = np.asarray(idx)
    wte = np.asarray(wte, f)
    wpe = np.asarray(wpe, f)
    x0_all = wte[idx] + wpe[None, :T]  # [8, T, E]

    attn_w = np.asarray(attn_w, f)[:nl]
    ln1_w = np.asarray(ln1_w, f)[:nl]
    fc_w = np.asarray(fc_w, f)[:nl]
    ln2_w = np.asarray(ln2_w, f)[:nl]
    proj_w = np.asarray(proj_w, f)[:nl]
    fc2_w = np.asarray(fc2_w, f)[:nl]

    # fold ln scale into following weights; fold sqrt(1/sqrt(D)) into W_q, W_k
    wqkv = attn_w * ln1_w[:, :, None]
    wqkv[:, :, : 2 * E] *= 1.0 / np.sqrt(np.sqrt(D))
    wfc = fc_w * ln2_w[:, :, None]

    # biases must be zero (true for this model)
    bqkv = np.einsum("le,lec->lc", np.asarray(ln1_b, f)[:nl], attn_w) + np.asarray(attn_b, f)[:nl]
    bfc = np.einsum("le,lec->lc", np.asarray(ln2_b, f)[:nl], fc_w) + np.asarray(fc_b, f)[:nl]
    for nm, b in [("bqkv", bqkv), ("proj_b", np.asarray(proj_b, f)),
                  ("bfc", bfc), ("fc2_b", np.asarray(fc2_b, f)),
                  ("lnf_b", np.asarray(lnf_b, f))]:
        assert np.abs(b).max() == 0.0, f"nonzero bias {nm} not supported by this kernel"

    def b16(w):
        return np.ascontiguousarray(w.astype(bf).view(np.uint16))

    # lhsT-style pack: W [nl, E, C] -> [nl, C/512, NEO, 128, 4, 128]
    def pack_l(w):
        ncol = w.shape[2] // 512
        x = w.reshape(nl, NEO, 128, ncol, 4, 128)  # [l, eo, p, q, ct, m]
        return b16(x.transpose(0, 3, 1, 2, 4, 5))  # [l, q, eo, p, ct, m]

    # rhs-style pack: W [nl, K, N] -> [nl, N/512, K/128, 128, 512]
    def pack_r(w):
        ncol = w.shape[2] // 512
        nk = w.shape[1] // 128
        x = w.reshape(nl, nk, 128, ncol, 512)  # [l, k, p, ch, n]
        return b16(x.transpose(0, 3, 1, 2, 4))  # [l, ch, k, p, n]

    wqk8 = pack_l(wqkv[:, :, : 2 * E])  # [l, 4, NEO, 128, 4, 128]; q=[Qg0,Qg1,Kg0,Kg1]
    wv8 = pack_r(wqkv[:, :, 2 * E :])  # [l, 2, NEO, 128, 512]
    wproj8 = pack_r(proj_w)
    # fc1: [l, 4E] cols -> slabs of 1024, 2 quarter-groups each
    wfcp = pack_l(wfc).reshape(nl, 4, 2, NEO, 128, 4, 128)
    # fc2: contraction 4E as 4 slabs x NEO; cols 1024 as 2x512
    x = fc2_w.reshape(nl, 4, NEO, 128, 2, 512)  # [l, slab, k, p, ch, n]
    wfc28 = b16(x.transpose(0, 1, 4, 2, 3, 5))  # [l, slab, ch, k, p, n]

    wtet = np.ascontiguousarray((wte * np.asarray(lnf_w, f)[None, :]).T)  # [E, V]
    shards = []
    for c in range(NCORES):
        sl = wtet[:, c * VSH : min(V, (c + 1) * VSH)]
        pad = np.zeros((E, VSP), f)
        pad[:, : sl.shape[1]] = sl
        shards.append(np.ascontiguousarray(pad.astype(bf).view(np.uint16)))

    return (
        np.ascontiguousarray(x0_all, f),
        wqk8, wv8, wproj8, wfcp, wfc28,
        shards,
    )


def kernel(idx, wte, wpe, ln1_w, ln1_b, attn_w, attn_b, proj_w, proj_b,
           ln2_w, ln2_b, fc_w, fc_b, fc2_w, fc2_b, lnf_w, lnf_b):
    import ml_dtypes
    from concourse.bass_utils import run_bass_kernel_spmd

    x0_all, wqk8, wv8, wproj8, wfc8, wfc28, shards = _host_prep(
        idx, wte, wpe, ln1_w, ln1_b, attn_w, attn_b, proj_w, proj_b,
        ln2_w, ln2_b, fc_w, fc_b, fc2_w, fc2_b, lnf_w, lnf_b, NL)

    if "p1" not in _CACHE:
        _CACHE["p1"] = _build_phase1(NL)
    nc1 = _CACHE["p1"]
    in_maps = [
        {"x0": x0_all[c], "wqk": wqk8, "wv": wv8, "wproj": wproj8,
         "wfc": wfc8, "wfc2": wfc28}
        for c in range(NCORES)
    ]
    trace = os.environ.get("GPT_TRACE", "0") == "1"
    r1 = run_bass_kernel_spmd(nc1, in_maps, core_ids=list(range(NCORES)), trace=trace)
    _CACHE["r1"] = r1
    xall = np.stack([r1.results[c]["xlast"][0] for c in range(NCORES)])  # [8, E]
    xallt = np.ascontiguousarray(
        xall.T.astype(ml_dtypes.bfloat16).view(np.uint16)
    )  # [E, 8] bf16

    if "p2" not in _CACHE:
        _CACHE["p2"] = _build_phase2()
    nc2 = _CACHE["p2"]
    in_maps2 = [{"xallt": xallt, "wtet": shards[c]} for c in range(NCORES)]
    r2 = run_bass_kernel_spmd(nc2, in_maps2, core_ids=list(range(NCORES)), trace=trace)
    _CACHE["r2"] = r2

    logits = np.zeros((NCORES, 1, V), np.float32)
    for c in range(NCORES):
        w = min(V, (c + 1) * VSH) - c * VSH
        logits[:, 0, c * VSH : c * VSH + w] = r2.results[c]["lg"][:, :w]
    return logits


# revision 10
# speedup vs baseline: 1.0916x; 1.0916x over previous
"""GPT-2 (L=12, E=1024, H=16, T=1024, B=8) forward on 8 Trainium2 NeuronCores.

Data-parallel over batch (1 sequence per core) for the 12 transformer layers;
vocab-parallel lm_head (each core computes a V/8 logits shard for all 8
sequences) as a second tiny NEFF, with the 8 last-position hidden vectors
gathered on host between the phases.

v3 (vs baseline f32r kernel):
  - all matmuls in bf16 (same PE rate as f32r but: half the DMA bytes,
    ldweights at 1 cycle/row, no 4x penalty on <256-wide moving chunks,
    transposes at 1 cycle/row)
  - softmax denominator folded into the AV matmul as a 65th ones-column of
    the V stationary tile (eliminates the separate PE denominator matmul)
  - per-token-tile layernorm pipelined behind proj / fc2-last-slab (PE never
    drains at sublayer boundaries, keeping the HAM clock warm)
  - scores/exp of head h software-pipelined with AV of head h-1
    (double-buffered attT)
  - weights host-packed into per-DMA-tile contiguous layouts (1KB/partition
    lines instead of 256-512B strided)

Host-side preprocessing (all linear folds, no model compute):
  - embedding gather x0 = wte[idx] + wpe  (pure indexing)
  - layernorm scale folded into the following matmul weights
  - sqrt(1/sqrt(D)) folded into both W_q and W_k
  - wte transposed (+ lnf scale) in bf16 for the lm_head
"""

import os
import sys

import numpy as np

sys.path.insert(0, "/opt/trn_rl_repo")

V, BLK, L, H, E = 50257, 1024, 12, 16, 1024
T = 1024
D = E // H  # 64
NCORES = 8
E3 = 3 * E
E4 = 4 * E
NTT = T // 128  # 8 token tiles
NEO = E // 128  # 8 embed tiles
VSH = (V + NCORES - 1) // NCORES  # 6283 vocab shard
VSP = 13 * 512  # 6656 padded shard width
NL = int(os.environ.get("GPT_NL", str(L)))

_CACHE = {}


def _build_phase1(nl):
    import concourse.mybir as mybir
    import concourse.tile as tile
    from concourse import bacc
    from concourse.masks import make_identity

    f32 = mybir.dt.float32
    bf = mybir.dt.bfloat16
    u16 = mybir.dt.uint16
    AF = mybir.ActivationFunctionType
    ALU = mybir.AluOpType

    nc = bacc.Bacc("TRN2", target_bir_lowering=False)

    x0 = nc.dram_tensor("x0", [T, E], f32, kind="ExternalInput")
    # bf16 weights as uint16 carriers (bitcast at DMA), packed per-DMA-tile:
    # lhsT-style [.., eo, p, ct, m]; rhs-style [.., k, p, n]
    wqk = nc.dram_tensor("wqk", [nl, 4, NEO, 128, 4, 128], u16, kind="ExternalInput")
    wv = nc.dram_tensor("wv", [nl, 2, NEO, 128, 512], u16, kind="ExternalInput")
    wproj = nc.dram_tensor("wproj", [nl, 2, NEO, 128, 512], u16, kind="ExternalInput")
    wfc = nc.dram_tensor("wfc", [nl, 4, 2, NEO, 128, 4, 128], u16, kind="ExternalInput")
    wfc2 = nc.dram_tensor("wfc2", [nl, 4, 2, NEO, 128, 512], u16, kind="ExternalInput")
    xlast = nc.dram_tensor("xlast", [1, E], f32, kind="ExternalOutput")

    with tile.TileContext(nc) as tc:
        import contextlib

        ctx = contextlib.ExitStack()
        with ctx:
            singles = ctx.enter_context(tc.tile_pool(name="singles", bufs=1))
            wl = ctx.enter_context(tc.tile_pool(name="wl", bufs=20))  # [128,4,128] bf
            wr = ctx.enter_context(tc.tile_pool(name="wr", bufs=26))  # [128,512] bf
            hpool = ctx.enter_context(tc.tile_pool(name="hpool", bufs=2))
            stat = ctx.enter_context(tc.tile_pool(name="stat", bufs=2))
            bc = ctx.enter_context(tc.tile_pool(name="bc", bufs=2))
            dram = ctx.enter_context(tc.tile_pool(name="dram", bufs=2, space="DRAM"))
            ps = ctx.enter_context(tc.tile_pool(name="ps", bufs=4, space="PSUM"))
            psb = ctx.enter_context(tc.tile_pool(name="psb", bufs=2, space="PSUM"))

            # ---- persistent tiles ----
            X = singles.tile([128, NTT, T], f32)  # residual [t, tt, e]
            HT = singles.tile([128, NEO, T], bf)  # ln-out transposed [e, eo, t]
            AOT = singles.tile([128, NEO, T], bf)  # attn outT [c, co, t]
            scrA = ctx.enter_context(tc.tile_pool(name="scrA", bufs=1))
            scrB = ctx.enter_context(tc.tile_pool(name="scrB", bufs=1))

            identb = singles.tile([128, 128], bf)
            make_identity(nc, identb)
            # maskT[k, q] = 0 if q >= k else -30  (additive, transposed causal)
            maskT = singles.tile([128, 128], f32)
            nc.gpsimd.memset(maskT, 0.0)
            nc.gpsimd.affine_select(
                out=maskT,
                in_=maskT,
                compare_op=ALU.is_ge,
                fill=-30.0,
                base=0,
                pattern=[[1, 128]],
                channel_multiplier=-1,
            )
            eps_t = singles.tile([128, 1], f32)
            nc.gpsimd.memset(eps_t, 1e-5)
            ones_row = singles.tile([1, 1024], f32)
            nc.gpsimd.memset(ones_row, 1.0)

            # ---- load x0 ----
            x0v = x0[:, :].rearrange("(tt p) e -> p tt e", p=128)
            for tt in range(NTT):
                nc.sync.dma_start(X[:, tt, :], x0v[:, tt, :])

            def ln_tt(src, tt):
                """LN(src[t,tt,:]) -> HT[:, :, tt*128:+128] bf16 via PE transposes."""
                st = stat.tile([128, 2, 6], f32, tag="bnst", name=f"st{tt}")
                for c in range(2):
                    nc.vector.bn_stats(st[:, c, :], src[:, tt, c * 512 : (c + 1) * 512])
                mv = stat.tile([128, 2], f32, tag="bnmv", name=f"mv{tt}")
                nc.vector.bn_aggr(mv, st)
                rstd = stat.tile([128, 1], f32, tag="rstd", name=f"rs{tt}")
                nc.scalar.activation(rstd, mv[:, 1:2], AF.Sqrt, bias=eps_t)
                nc.vector.reciprocal(rstd, rstd)
                h = hpool.tile([128, T], bf, tag="h", name=f"h{tt}")
                nc.vector.tensor_scalar(
                    out=h,
                    in0=src[:, tt, :],
                    scalar1=mv[:, 0:1],
                    scalar2=rstd,
                    op0=ALU.subtract,
                    op1=ALU.mult,
                )
                for half in range(2):
                    ptr = ps.tile([128, 4, 128], bf, tag="mm", name=f"ptr{tt}_{half}")
                    for eq in range(4):
                        eo = half * 4 + eq
                        nc.tensor.transpose(ptr[:, eq, :], h[:, eo * 128 : (eo + 1) * 128], identb)
                    nc.scalar.activation(
                        HT[:, half * 4 : half * 4 + 4, tt * 128 : (tt + 1) * 128],
                        ptr,
                        AF.Copy,
                    )

            def mm_lhsw_to_ct(dst, wdram_l, n_ct, act=AF.Copy):
                """dst[:, ct, t] (bf16) = W[ct].T @ HT for ct in range(n_ct).
                wdram_l: [NEO, 128, 4, 128] packed quarter (n_ct=4) or
                list of two for n_ct=8."""
                quarters = wdram_l if isinstance(wdram_l, list) else [wdram_l]
                for qi, wq in enumerate(quarters):
                    wts = []
                    for eo in range(NEO):
                        wt = wl.tile([128, 4, 128], bf, tag="w_l", name=f"wl{qi}_{eo}")
                        nc.sync.dma_start(wt, wq[eo].bitcast(bf))
                        wts.append(wt)
                    for ct in range(4):
                        for ch in range(2):
                            pt = ps.tile([128, 512], f32, tag="mm", name=f"p{qi}_{ct}_{ch}")
                            for eo in range(NEO):
                                nc.tensor.matmul(
                                    pt,
                                    wts[eo][:, ct, :],
                                    HT[:, eo, ch * 512 : (ch + 1) * 512],
                                    start=(eo == 0),
                                    stop=(eo == NEO - 1),
                                )
                            nc.scalar.activation(
                                dst[:, qi * 4 + ct, ch * 512 : (ch + 1) * 512], pt, act
                            )

            def rhs_group(lhsT3, wts, tt, name):
                pt = ps.tile([128, 512], f32, tag="mm", name=name)
                for k in range(NEO):
                    nc.tensor.matmul(
                        pt,
                        lhsT3[:, k, tt * 128 : (tt + 1) * 128],
                        wts[k],
                        start=(k == 0),
                        stop=(k == NEO - 1),
                    )
                return pt

            def load_wr(wdram, n, name):
                wts = []
                for k in range(n):
                    wt = wr.tile([128, 512], bf, tag="w_r", name=f"{name}{k}")
                    nc.sync.dma_start(wt, wdram[k].bitcast(bf))
                    wts.append(wt)
                return wts

            for l in range(nl):
                # ===== attention =====
                if l == 0:
                    for tt in range(NTT):
                        ln_tt(X, tt)
                for g in range(2):  # head groups of 8 heads (512 c-cols)
                    qkg = scrA.tile([128, 8, T], bf, tag="scrA", name=f"qkg{l}_{g}")
                    vatt = scrB.tile([128, 20544], bf, tag="scrB", name=f"vatt{l}_{g}")
                    QG = qkg[:, 0:4, :]
                    KG = qkg[:, 4:8, :]
                    # V with a 65th ones-column per head (softmax denom trick)
                    VG = vatt[:, 0:4160].rearrange("p (j h c) -> p j h c", j=NTT, h=8)
                    # double-buffered non-ragged attT [parity, j, q]
                    attT = vatt[:, 4160:20544].rearrange("p (b j q) -> p b j q", b=2, j=NTT)
                    mm_lhsw_to_ct(QG, wqk[l, g], 4)
                    mm_lhsw_to_ct(KG, wqk[l, 2 + g], 4)

                    vw = load_wr(wv[l, g], NEO, f"vw{g}")
                    for tt in range(NTT):
                        pt = rhs_group(HT, vw, tt, f"pv{tt}")
                        nc.scalar.activation(VG[:, tt, :, 0:64], pt, AF.Copy)
                    nc.gpsimd.memset(VG[:, :, :, 64:65], 1.0)

                    def scores(hh):
                        pb = hh % 2
                        ct, ro = hh // 2, (hh % 2) * 64
                        qT = QG[ro : ro + 64, ct, :]  # [64, 1024] bf16
                        kT = KG[ro : ro + 64, ct, :]
                        for j in range(NTT):
                            q0 = j * 128
                            for ch in range(q0, T, 512):
                                w = min(512, T - ch)
                                pa = ps.tile([128, 512], f32, tag="mm", name=f"pa{j}_{ch}")
                                nc.tensor.matmul(
                                    pa[:, :w],
                                    kT[:, q0 : q0 + 128],
                                    qT[:, ch : ch + w],
                                    start=True,
                                    stop=True,
                                )
                                if ch == q0:  # causal mask on diagonal block
                                    nc.vector.tensor_tensor(pa[:, :128], pa[:, :128], maskT, ALU.add)
                                nc.scalar.activation(attT[:, pb, j, ch : ch + w], pa[:, :w], AF.Exp)

                    def av(hh):
                        pb = hh % 2
                        h_glob = g * 8 + hh
                        av_ps = psb.tile([65, 1024], f32, tag="av", name=f"av{l}_{h_glob}")
                        for j in range(NTT):
                            vsl = VG[:, j, hh, 0:65]
                            for ca in range(2):
                                s = max(ca * 512, j * 128)
                                if s >= (ca + 1) * 512:
                                    continue
                                w = (ca + 1) * 512 - s
                                nc.tensor.matmul(
                                    av_ps[:, s : s + w],
                                    vsl,
                                    attT[:, pb, j, s : s + w],
                                    start=(j == 0),
                                    stop=(j == (3 if ca == 0 else 7)),
                                    skip_group_check=True,
                                )
                        # denominator: evac row 64 (ScalarE), DMA-spread to
                        # [128, 8] so the DVE reciprocal is 8 els/lane, DMA
                        # back and broadcast-read to 64 partitions
                        dsb = stat.tile([1, 1024], f32, tag="dsb", name=f"ds{l}_{h_glob}")
                        nc.scalar.activation(dsb, av_ps[64:65, :], AF.Copy)
                        dd = dram.tile([1, 1024], f32, tag="dd", name=f"dd{l}_{h_glob}")
                        nc.sync.dma_start(dd, dsb)
                        sp = stat.tile([128, 8], f32, tag="sp", name=f"sp{l}_{h_glob}")
                        nc.sync.dma_start(sp, dd[0, :].rearrange("(p i) -> p i", p=128))
                        nc.vector.reciprocal(sp, sp)
                        dd2 = dram.tile([1, 1024], f32, tag="dd2", name=f"d2{l}_{h_glob}")
                        nc.sync.dma_start(dd2[0, :].rearrange("(p i) -> p i", p=128), sp)
                        den64 = bc.tile([64, 1024], f32, tag="rdb", name=f"dn{l}_{h_glob}")
                        nc.gpsimd.dma_start(den64, dd2.to_broadcast([64, 1024]))
                        co, ro2 = h_glob // 2, (h_glob % 2) * 64
                        nc.vector.tensor_tensor(
                            AOT[ro2 : ro2 + 64, co, :], av_ps[0:64, :], den64, ALU.mult
                        )

                    for hh in range(8):
                        scores(hh)
                        if hh > 0:
                            av(hh - 1)
                    av(7)

                # proj + residual, mlp-LN pipelined per token tile
                pw = load_wr(wproj[l, 0], NEO, "pw0") + load_wr(wproj[l, 1], NEO, "pw1")
                for tt in range(NTT):
                    for ch in range(2):
                        pt = rhs_group(AOT, pw[ch * NEO : (ch + 1) * NEO], tt, f"pp{tt}_{ch}")
                        nc.vector.tensor_tensor(
                            X[:, tt, ch * 512 : (ch + 1) * 512],
                            X[:, tt, ch * 512 : (ch + 1) * 512],
                            pt,
                            ALU.add,
                        )
                    ln_tt(X, tt)

                # ===== mlp =====
                FC2A = scrB.tile([128, NTT, T], f32, tag="scrB", name=f"fc2a{l}")
                for slab in range(4):  # 4E in 4 slabs of 1024
                    H1T = scrA.tile([128, 8, T], bf, tag="scrA", name=f"h1t{l}_{slab}")
                    mm_lhsw_to_ct(
                        H1T, [wfc[l, slab, 0], wfc[l, slab, 1]], 8, act=AF.Gelu_apprx_tanh
                    )
                    f2w = load_wr(wfc2[l, slab, 0], NEO, f"f2a{slab}") + load_wr(
                        wfc2[l, slab, 1], NEO, f"f2b{slab}"
                    )
                    last = slab == 3
                    for tt in range(NTT):
                        for ch in range(2):
                            pt = rhs_group(H1T, f2w[ch * NEO : (ch + 1) * NEO], tt, f"pf{slab}_{tt}_{ch}")
                            if slab == 0:
                                nc.vector.tensor_tensor(
                                    FC2A[:, tt, ch * 512 : (ch + 1) * 512],
                                    X[:, tt, ch * 512 : (ch + 1) * 512],
                                    pt,
                                    ALU.add,
                                )
                            elif not last:
                                nc.vector.tensor_tensor(
                                    FC2A[:, tt, ch * 512 : (ch + 1) * 512],
                                    FC2A[:, tt, ch * 512 : (ch + 1) * 512],
                                    pt,
                                    ALU.add,
                                )
                            else:
                                nc.vector.tensor_tensor(
                                    X[:, tt, ch * 512 : (ch + 1) * 512],
                                    FC2A[:, tt, ch * 512 : (ch + 1) * 512],
                                    pt,
                                    ALU.add,
                                )
                        if last and l + 1 < nl:
                            ln_tt(X, tt)

            # ===== final layernorm on last token tile, emit last row =====
            st = stat.tile([128, 2, 6], f32, tag="bnst", name="stf")
            for c in range(2):
                nc.vector.bn_stats(st[:, c, :], X[:, NTT - 1, c * 512 : (c + 1) * 512])
            mv = stat.tile([128, 2], f32, tag="bnmv", name="mvf")
            nc.vector.bn_aggr(mv, st)
            rstd = stat.tile([128, 1], f32, tag="rstd", name="rsf")
            nc.scalar.activation(rstd, mv[:, 1:2], AF.Sqrt, bias=eps_t)
            nc.vector.reciprocal(rstd, rstd)
            xn = hpool.tile([128, T], f32, tag="xn", name="xnf")
            nc.vector.tensor_scalar(
                out=xn,
                in0=X[:, NTT - 1, :],
                scalar1=mv[:, 0:1],
                scalar2=rstd,
                op0=ALU.subtract,
                op1=ALU.mult,
            )
            nc.sync.dma_start(xlast[:, :], xn[127:128, :])

    nc.compile()
    return nc


def _build_phase2():
    import concourse.mybir as mybir
    import concourse.tile as tile
    from concourse import bacc

    f32 = mybir.dt.float32
    bf = mybir.dt.bfloat16
    u16 = mybir.dt.uint16
    AF = mybir.ActivationFunctionType

    nc = bacc.Bacc("TRN2", target_bir_lowering=False)
    xallt = nc.dram_tensor("xallt", [E, NCORES], u16, kind="ExternalInput")
    wtet = nc.dram_tensor("wtet", [E, VSP], u16, kind="ExternalInput")
    lg = nc.dram_tensor("lg", [NCORES, VSP], f32, kind="ExternalOutput")

    with tile.TileContext(nc) as tc:
        with (
            tc.tile_pool(name="s", bufs=1) as s,
            tc.tile_pool(name="w", bufs=6) as w,
            tc.tile_pool(name="o", bufs=4) as o,
            tc.tile_pool(name="p", bufs=4, space="PSUM") as p,
        ):
            xt = s.tile([128, NEO, NCORES], bf)
            nc.sync.dma_start(xt, xallt[:, :].rearrange("(eo p) s -> p eo s", p=128).bitcast(bf))
            for vc in range(VSP // 512):
                pt = p.tile([NCORES, 512], f32, tag="p", name=f"p{vc}")
                for eo in range(NEO):
                    wt = w.tile([128, 512], bf, tag="w", name=f"w{vc}_{eo}")
                    nc.sync.dma_start(
                        wt, wtet[eo * 128 : (eo + 1) * 128, vc * 512 : (vc + 1) * 512].bitcast(bf)
                    )
                    nc.tensor.matmul(pt, xt[:, eo, :], wt, start=(eo == 0), stop=(eo == NEO - 1))
                ot = o.tile([NCORES, 512], f32, tag="o", name=f"o{vc}")
                nc.scalar.activation(ot, pt, AF.Copy)
                nc.sync.dma_start(lg[:, vc * 512 : (vc + 1) * 512], ot)
    nc.compile()
    return nc


def _host_prep(idx, wte, wpe, ln1_w, ln1_b, attn_w, attn_b, proj_w, proj_b,
               ln2_w, ln2_b, fc_w, fc_b, fc2_w, fc2_b, lnf_w, lnf_b, nl):
    import ml_dtypes

    bf = ml_dtypes.bfloat16
    f = np.float32
    idx = np.asarray(idx)
    wte = np.asarray(wte, f)
    wpe = np.asarray(wpe, f)
    x0_all = wte[idx] + wpe[None, :T]  # [8, T, E]

    attn_w = np.asarray(attn_w, f)[:nl]
    ln1_w = np.asarray(ln1_w, f)[:nl]
    fc_w = np.asarray(fc_w, f)[:nl]
    ln2_w = np.asarray(ln2_w, f)[:nl]
    proj_w = np.asarray(proj_w, f)[:nl]
    fc2_w = np.asarray(fc2_w, f)[:nl]

    # fold ln scale into following weights; fold sqrt(1/sqrt(D)) into W_q, W_k
    wqkv = attn_w * ln1_w[:, :, None]
    wqkv[:, :, : 2 * E] *= 1.0 / np.sqrt(np.sqrt(D))
    wfc = fc_w * ln2_w[:, :, None]

    # biases must be zero (true for this model)
    bqkv = np.einsum("le,lec->lc", np.asarray(ln1_b, f)[:nl], attn_w) + np.asarray(attn_b, f)[:nl]
    bfc = np.einsum("le,lec->lc", np.asarray(ln2_b, f)[:nl], fc_w) + np.asarray(fc_b, f)[:nl]
    for nm, b in [("bqkv", bqkv), ("proj_b", np.asarray(proj_b, f)),
                  ("bfc", bfc), ("fc2_b", np.asarray(fc2_b, f)),
                  ("lnf_b", np.asarray(lnf_b, f))]:
        assert np.abs(b).max() == 0.0, f"nonzero bias {nm} not supported by this kernel"

    def b16(w):
        return np.ascontiguousarray(w.astype(bf).view(np.uint16))

    # lhsT-style pack: W [nl, E, C] -> [nl, C/512, NEO, 128, 4, 128]
    def pack_l(w):
        ncol = w.shape[2] // 512
        x = w.reshape(nl, NEO, 128, ncol, 4, 128)  # [l, eo, p, q, ct, m]
        return b16(x.transpose(0, 3, 1, 2, 4, 5))  # [l, q, eo, p, ct, m]

    # rhs-style pack: W [nl, K, N] -> [nl, N/512, K/128, 128, 512]
    def pack_r(w):
        ncol = w.shape[2] // 512
        nk = w.shape[1] // 128
        x = w.reshape(nl, nk, 128, ncol, 512)  # [l, k, p, ch, n]
        return b16(x.transpose(0, 3, 1, 2, 4))  # [l, ch, k, p, n]

    wqk8 = pack_l(wqkv[:, :, : 2 * E])  # [l, 4, NEO, 128, 4, 128]; q=[Qg0,Qg1,Kg0,Kg1]
    wv8 = pack_r(wqkv[:, :, 2 * E :])  # [l, 2, NEO, 128, 512]
    wproj8 = pack_r(proj_w)
    # fc1: [l, 4E] cols -> slabs of 1024, 2 quarter-groups each
    wfcp = pack_l(wfc).reshape(nl, 4, 2, NEO, 128, 4, 128)
    # fc2: contraction 4E as 4 slabs x NEO; cols 1024 as 2x512
    x = fc2_w.reshape(nl, 4, NEO, 128, 2, 512)  # [l, slab, k, p, ch, n]
    wfc28 = b16(x.transpose(0, 1, 4, 2, 3, 5))  # [l, slab, ch, k, p, n]

    wtet = np.ascontiguousarray((wte * np.asarray(lnf_w, f)[None, :]).T)  # [E, V]
    shards = []
    for c in range(NCORES):
        sl = wtet[:, c * VSH : min(V, (c + 1) * VSH)]
        pad = np.zeros((E, VSP), f)
        pad[:, : sl.shape[1]] = sl
        shards.append(np.ascontiguousarray(pad.astype(bf).view(np.uint16)))

    return (
        np.ascontiguousarray(x0_all, f),
        wqk8, wv8, wproj8, wfcp, wfc28,
        shards,
    )


def kernel(idx, wte, wpe, ln1_w, ln1_b, attn_w, attn_b, proj_w, proj_b,
           ln2_w, ln2_b, fc_w, fc_b, fc2_w, fc2_b, lnf_w, lnf_b):
    import ml_dtypes
    from concourse.bass_utils import run_bass_kernel_spmd

    x0_all, wqk8, wv8, wproj8, wfc8, wfc28, shards = _host_prep(
        idx, wte, wpe, ln1_w, ln1_b, attn_w, attn_b, proj_w, proj_b,
        ln2_w, ln2_b, fc_w, fc_b, fc2_w, fc2_b, lnf_w, lnf_b, NL)

    if "p1" not in _CACHE:
        _CACHE["p1"] = _build_phase1(NL)
    nc1 = _CACHE["p1"]
    in_maps = [
        {"x0": x0_all[c], "wqk": wqk8, "wv": wv8, "wproj": wproj8,
         "wfc": wfc8, "wfc2": wfc28}
        for c in range(NCORES)
    ]
    trace = os.environ.get("GPT_TRACE", "0") == "1"
    r1 = run_bass_kernel_spmd(nc1, in_maps, core_ids=list(range(NCORES)), trace=trace)
    _CACHE["r1"] = r1
    xall = np.stack([r1.results[c]["xlast"][0] for c in range(NCORES)])  # [8, E]
    xallt = np.ascontiguousarray(
        xall.T.astype(ml_dtypes.bfloat16).view(np.uint16)
    )  # [E, 8] bf16

    if "p2" not in _CACHE:
        _CACHE["p2"] = _build_phase2()
    nc2 = _CACHE["p2"]
    in_maps2 = [{"xallt": xallt, "wtet": shards[c]} for c in range(NCORES)]
    r2 = run_bass_kernel_spmd(nc2, in_maps2, core_ids=list(range(NCORES)), trace=trace)
    _CACHE["r2"] = r2

    logits = np.zeros((NCORES, 1, V), np.float32)
    for c in range(NCORES):
        w = min(V, (c + 1) * VSH) - c * VSH
        logits[:, 0, c * VSH : c * VSH + w] = r2.results[c]["lg"][:, :w]
    return logits


# revision 13
# speedup vs baseline: 1.1312x; 1.0363x over previous
"""GPT-2 (L=12, E=1024, H=16, T=1024, B=8) forward on 8 Trainium2 NeuronCores.

Data-parallel over batch (1 sequence per core) for the 12 transformer layers;
vocab-parallel lm_head (each core computes a V/8 logits shard for all 8
sequences) as a second tiny NEFF, with the 8 last-position hidden vectors
gathered on host between the phases.

v3 (vs baseline f32r kernel):
  - all matmuls in bf16 (same PE rate as f32r but: half the DMA bytes,
    ldweights at 1 cycle/row, no 4x penalty on <256-wide moving chunks,
    transposes at 1 cycle/row)
  - softmax denominator folded into the AV matmul as a 65th ones-column of
    the V stationary tile (eliminates the separate PE denominator matmul)
  - per-token-tile layernorm pipelined behind proj / fc2-last-slab (PE never
    drains at sublayer boundaries, keeping the HAM clock warm)
  - scores/exp of head h software-pipelined with AV of head h-1
    (double-buffered attT)
  - weights host-packed into per-DMA-tile contiguous layouts (1KB/partition
    lines instead of 256-512B strided)

Host-side preprocessing (all linear folds, no model compute):
  - embedding gather x0 = wte[idx] + wpe  (pure indexing)
  - layernorm scale folded into the following matmul weights
  - sqrt(1/sqrt(D)) folded into both W_q and W_k
  - wte transposed (+ lnf scale) in bf16 for the lm_head
"""

import os
import sys

import numpy as np

sys.path.insert(0, "/opt/trn_rl_repo")

V, BLK, L, H, E = 50257, 1024, 12, 16, 1024
T = 1024
D = E // H  # 64
NCORES = 8
E3 = 3 * E
E4 = 4 * E
NTT = T // 128  # 8 token tiles
NEO = E // 128  # 8 embed tiles
VSH = (V + NCORES - 1) // NCORES  # 6283 vocab shard
VSP = 13 * 512  # 6656 padded shard width
NL = int(os.environ.get("GPT_NL", str(L)))

_CACHE = {}


def _build_phase1(nl):
    import concourse.mybir as mybir
    import concourse.tile as tile
    from concourse import bacc
    from concourse.masks import make_identity

    f32 = mybir.dt.float32
    bf = mybir.dt.bfloat16
    u16 = mybir.dt.uint16
    AF = mybir.ActivationFunctionType
    ALU = mybir.AluOpType

    nc = bacc.Bacc("TRN2", target_bir_lowering=False)

    x0 = nc.dram_tensor("x0", [T, E], f32, kind="ExternalInput")
    # bf16 weights as uint16 carriers (bitcast at DMA), packed per-DMA-tile:
    # lhsT-style [.., eo, p, ct, m]; rhs-style [.., k, p, n]
    wqk = nc.dram_tensor("wqk", [nl, 4, NEO, 128, 4, 128], u16, kind="ExternalInput")
    wv = nc.dram_tensor("wv", [nl, 2, NEO, 128, 512], u16, kind="ExternalInput")
    wproj = nc.dram_tensor("wproj", [nl, 2, NEO, 128, 512], u16, kind="ExternalInput")
    wfc = nc.dram_tensor("wfc", [nl, 4, 2, NEO, 128, 4, 128], u16, kind="ExternalInput")
    wfc2 = nc.dram_tensor("wfc2", [nl, 4, 2, NEO, 128, 512], u16, kind="ExternalInput")
    xlast = nc.dram_tensor("xlast", [1, E], f32, kind="ExternalOutput")

    with tile.TileContext(nc) as tc:
        import contextlib

        ctx = contextlib.ExitStack()
        with ctx:
            singles = ctx.enter_context(tc.tile_pool(name="singles", bufs=1))
            wl = ctx.enter_context(tc.tile_pool(name="wl", bufs=20))  # [128,4,128] bf
            wr = ctx.enter_context(tc.tile_pool(name="wr", bufs=26))  # [128,512] bf
            hpool = ctx.enter_context(tc.tile_pool(name="hpool", bufs=2))
            stat = ctx.enter_context(tc.tile_pool(name="stat", bufs=2))
            bc = ctx.enter_context(tc.tile_pool(name="bc", bufs=2))
            dram = ctx.enter_context(tc.tile_pool(name="dram", bufs=2, space="DRAM"))
            ps = ctx.enter_context(tc.tile_pool(name="ps", bufs=4, space="PSUM"))
            psb = ctx.enter_context(tc.tile_pool(name="psb", bufs=2, space="PSUM"))

            # ---- persistent tiles ----
            X = singles.tile([128, NTT, T], f32)  # residual [t, tt, e]
            HT = singles.tile([128, NEO, T], bf)  # ln-out transposed [e, eo, t]
            AOT = singles.tile([128, NEO, T], bf)  # attn outT [c, co, t]
            scrA = ctx.enter_context(tc.tile_pool(name="scrA", bufs=1))
            scrB = ctx.enter_context(tc.tile_pool(name="scrB", bufs=1))

            identb = singles.tile([128, 128], bf)
            make_identity(nc, identb)
            # maskT[k, q] = 0 if q >= k else -30  (additive, transposed causal)
            maskT = singles.tile([128, 128], f32)
            nc.gpsimd.memset(maskT, 0.0)
            nc.gpsimd.affine_select(
                out=maskT,
                in_=maskT,
                compare_op=ALU.is_ge,
                fill=-30.0,
                base=0,
                pattern=[[1, 128]],
                channel_multiplier=-1,
            )
            eps_t = singles.tile([128, 1], f32)
            nc.gpsimd.memset(eps_t, 1e-5)
            ones_row = singles.tile([1, 1024], f32)
            nc.gpsimd.memset(ones_row, 1.0)

            # ---- load x0 ----
            x0v = x0[:, :].rearrange("(tt p) e -> p tt e", p=128)
            for tt in range(NTT):
                nc.sync.dma_start(X[:, tt, :], x0v[:, tt, :])

            def ln_tt(src, tt):
                """LN(src[t,tt,:]) -> HT[:, :, tt*128:+128] bf16 via PE transposes."""
                st = stat.tile([128, 2, 6], f32, tag="bnst", name=f"st{tt}")
                for c in range(2):
                    nc.vector.bn_stats(st[:, c, :], src[:, tt, c * 512 : (c + 1) * 512])
                mv = stat.tile([128, 2], f32, tag="bnmv", name=f"mv{tt}")
                nc.vector.bn_aggr(mv, st)
                rstd = stat.tile([128, 1], f32, tag="rstd", name=f"rs{tt}")
                nc.scalar.activation(rstd, mv[:, 1:2], AF.Sqrt, bias=eps_t)
                nc.vector.reciprocal(rstd, rstd)
                h = hpool.tile([128, T], bf, tag="h", name=f"h{tt}")
                nc.vector.tensor_scalar(
                    out=h,
                    in0=src[:, tt, :],
                    scalar1=mv[:, 0:1],
                    scalar2=rstd,
                    op0=ALU.subtract,
                    op1=ALU.mult,
                )
                for half in range(2):
                    ptr = ps.tile([128, 4, 128], bf, tag="mm", name=f"ptr{tt}_{half}")
                    for eq in range(4):
                        eo = half * 4 + eq
                        nc.tensor.transpose(ptr[:, eq, :], h[:, eo * 128 : (eo + 1) * 128], identb)
                    nc.scalar.activation(
                        HT[:, half * 4 : half * 4 + 4, tt * 128 : (tt + 1) * 128],
                        ptr,
                        AF.Copy,
                    )

            def mm_lhsw_to_ct(dst, wdram_l, n_ct, act=AF.Copy):
                """dst[:, ct, t] (bf16) = W[ct].T @ HT for ct in range(n_ct).
                wdram_l: [NEO, 128, 4, 128] packed quarter (n_ct=4) or
                list of two for n_ct=8."""
                quarters = wdram_l if isinstance(wdram_l, list) else [wdram_l]
                for qi, wq in enumerate(quarters):
                    wts = []
                    for eo in range(NEO):
                        wt = wl.tile([128, 4, 128], bf, tag="w_l", name=f"wl{qi}_{eo}")
                        nc.sync.dma_start(wt, wq[eo].bitcast(bf))
                        wts.append(wt)
                    for ct in range(4):
                        for ch in range(2):
                            pt = ps.tile([128, 512], f32, tag="mm", name=f"p{qi}_{ct}_{ch}")
                            for eo in range(NEO):
                                nc.tensor.matmul(
                                    pt,
                                    wts[eo][:, ct, :],
                                    HT[:, eo, ch * 512 : (ch + 1) * 512],
                                    start=(eo == 0),
                                    stop=(eo == NEO - 1),
                                )
                            nc.scalar.activation(
                                dst[:, qi * 4 + ct, ch * 512 : (ch + 1) * 512], pt, act
                            )

            def rhs_group(lhsT3, wts, tt, name):
                pt = ps.tile([128, 512], f32, tag="mm", name=name)
                for k in range(NEO):
                    nc.tensor.matmul(
                        pt,
                        lhsT3[:, k, tt * 128 : (tt + 1) * 128],
                        wts[k],
                        start=(k == 0),
                        stop=(k == NEO - 1),
                    )
                return pt

            def load_wr(wdram, n, name):
                wts = []
                for k in range(n):
                    wt = wr.tile([128, 512], bf, tag="w_r", name=f"{name}{k}")
                    nc.sync.dma_start(wt, wdram[k].bitcast(bf))
                    wts.append(wt)
                return wts

            for l in range(nl):
                # ===== attention =====
                if l == 0:
                    for tt in range(NTT):
                        ln_tt(X, tt)
                for g in range(2):  # head groups of 8 heads (512 c-cols)
                    qkg = scrA.tile([128, 8, T], bf, tag="scrA", name=f"qkg{l}_{g}")
                    vatt = scrB.tile([128, 20544], bf, tag="scrB", name=f"vatt{l}_{g}")
                    QG = qkg[:, 0:4, :]
                    KG = qkg[:, 4:8, :]
                    # V with a 65th ones-column per head (softmax denom trick)
                    VG = vatt[:, 0:4160].rearrange("p (j h c) -> p j h c", j=NTT, h=8)
                    # double-buffered non-ragged attT [parity, j, q]
                    attT = vatt[:, 4160:20544].rearrange("p (b j q) -> p b j q", b=2, j=NTT)
                    mm_lhsw_to_ct(QG, wqk[l, g], 4)
                    mm_lhsw_to_ct(KG, wqk[l, 2 + g], 4)

                    vw = load_wr(wv[l, g], NEO, f"vw{g}")
                    for tt in range(NTT):
                        pt = rhs_group(HT, vw, tt, f"pv{tt}")
                        nc.scalar.activation(VG[:, tt, :, 0:64], pt, AF.Copy)
                    nc.gpsimd.memset(VG[:, :, :, 64:65], 1.0)

                    def scores(hh):
                        pb = hh % 2
                        ct, ro = hh // 2, (hh % 2) * 64
                        qT = QG[ro : ro + 64, ct, :]  # [64, 1024] bf16
                        kT = KG[ro : ro + 64, ct, :]
                        for j in range(NTT):
                            q0 = j * 128
                            for ch in range(q0, T, 512):
                                w = min(512, T - ch)
                                pa = ps.tile([128, 512], f32, tag="mm", name=f"pa{j}_{ch}")
                                nc.tensor.matmul(
                                    pa[:, :w],
                                    kT[:, q0 : q0 + 128],
                                    qT[:, ch : ch + w],
                                    start=True,
                                    stop=True,
                                )
                                if ch == q0:  # causal mask on diagonal block
                                    nc.vector.tensor_tensor(pa[:, :128], pa[:, :128], maskT, ALU.add)
                                nc.scalar.activation(attT[:, pb, j, ch : ch + w], pa[:, :w], AF.Exp)

                    def av(hh):
                        pb = hh % 2
                        h_glob = g * 8 + hh
                        av_ps = psb.tile([65, 1024], f32, tag="av", name=f"av{l}_{h_glob}")
                        for j in range(NTT):
                            vsl = VG[:, j, hh, 0:65]
                            for ca in range(2):
                                s = max(ca * 512, j * 128)
                                if s >= (ca + 1) * 512:
                                    continue
                                w = (ca + 1) * 512 - s
                                nc.tensor.matmul(
                                    av_ps[:, s : s + w],
                                    vsl,
                                    attT[:, pb, j, s : s + w],
                                    start=(j == 0),
                                    stop=(j == (3 if ca == 0 else 7)),
                                    skip_group_check=True,
                                )
                        # free av_ps fast: unnormalized AV -> SBUF (bf16)
                        avu = bc.tile([64, 1024], bf, tag="avu", bufs=4, name=f"au{l}_{h_glob}")
                        nc.vector.tensor_scalar(
                            out=avu, in0=av_ps[0:64, :], scalar1=1.0, scalar2=None,
                            op0=ALU.mult,
                        )
                        # denominator: evac row 64 (ScalarE), DMA-spread to
                        # [128, 8] so the DVE reciprocal is 8 els/lane, DMA
                        # back and broadcast-read to 64 partitions
                        dsb = stat.tile([1, 1024], f32, tag="dsb", name=f"ds{l}_{h_glob}")
                        nc.scalar.activation(dsb, av_ps[64:65, :], AF.Copy)
                        dd = dram.tile([1, 1024], f32, tag="dd", name=f"dd{l}_{h_glob}")
                        nc.sync.dma_start(dd, dsb)
                        sp = stat.tile([128, 8], f32, tag="sp", name=f"sp{l}_{h_glob}")
                        nc.sync.dma_start(sp, dd[0, :].rearrange("(p i) -> p i", p=128))
                        nc.vector.reciprocal(sp, sp)
                        spb = stat.tile([128, 8], bf, tag="spb", name=f"sb{l}_{h_glob}")
                        nc.vector.tensor_scalar(
                            out=spb, in0=sp, scalar1=1.0, scalar2=None, op0=ALU.mult
                        )
                        dd2 = dram.tile([1, 1024], bf, tag="dd2", name=f"d2{l}_{h_glob}")
                        nc.sync.dma_start(dd2[0, :].rearrange("(p i) -> p i", p=128), spb)
                        den64 = bc.tile([64, 1024], bf, tag="rdb", name=f"dn{l}_{h_glob}")
                        nc.gpsimd.dma_start(den64, dd2.to_broadcast([64, 1024]))
                        co, ro2 = h_glob // 2, (h_glob % 2) * 64
                        nc.vector.tensor_tensor(
                            AOT[ro2 : ro2 + 64, co, :], avu, den64, ALU.mult
                        )

                    for hh in range(8):
                        scores(hh)
                        if hh > 0:
                            av(hh - 1)
                    av(7)

                # proj + residual, mlp-LN pipelined per token tile
                pw = load_wr(wproj[l, 0], NEO, "pw0") + load_wr(wproj[l, 1], NEO, "pw1")
                for tt in range(NTT):
                    for ch in range(2):
                        pt = rhs_group(AOT, pw[ch * NEO : (ch + 1) * NEO], tt, f"pp{tt}_{ch}")
                        nc.vector.tensor_tensor(
                            X[:, tt, ch * 512 : (ch + 1) * 512],
                            X[:, tt, ch * 512 : (ch + 1) * 512],
                            pt,
                            ALU.add,
                        )
                    ln_tt(X, tt)

                # ===== mlp =====
                FC2A = scrB.tile([128, NTT, T], f32, tag="scrB", name=f"fc2a{l}")
                for slab in range(4):  # 4E in 4 slabs of 1024
                    H1T = scrA.tile([128, 8, T], bf, tag="scrA", name=f"h1t{l}_{slab}")
                    mm_lhsw_to_ct(
                        H1T, [wfc[l, slab, 0], wfc[l, slab, 1]], 8, act=AF.Gelu_apprx_tanh
                    )
                    f2w = load_wr(wfc2[l, slab, 0], NEO, f"f2a{slab}") + load_wr(
                        wfc2[l, slab, 1], NEO, f"f2b{slab}"
                    )
                    last = slab == 3
                    for tt in range(NTT):
                        for ch in range(2):
                            pt = rhs_group(H1T, f2w[ch * NEO : (ch + 1) * NEO], tt, f"pf{slab}_{tt}_{ch}")
                            if slab == 0:
                                nc.vector.tensor_tensor(
                                    FC2A[:, tt, ch * 512 : (ch + 1) * 512],
                                    X[:, tt, ch * 512 : (ch + 1) * 512],
                                    pt,
                                    ALU.add,
                                )
                            elif not last:
                                nc.vector.tensor_tensor(
                                    FC2A[:, tt, ch * 512 : (ch + 1) * 512],
                                    FC2A[:, tt, ch * 512 : (ch + 1) * 512],
                                    pt,
                                    ALU.add,
                                )
                            else:
                                nc.vector.tensor_tensor(
                                    X[:, tt, ch * 512 : (ch + 1) * 512],
                                    FC2A[:, tt, ch * 512 : (ch + 1) * 512],
                                    pt,
                                    ALU.add,
                                )
                        if last and l + 1 < nl:
                            ln_tt(X, tt)

            # ===== final layernorm on last token tile, emit last row =====
            st = stat.tile([128, 2, 6], f32, tag="bnst", name="stf")
            for c in range(2):
                nc.vector.bn_stats(st[:, c, :], X[:, NTT - 1, c * 512 : (c + 1) * 512])
            mv = stat.tile([128, 2], f32, tag="bnmv", name="mvf")
            nc.vector.bn_aggr(mv, st)
            rstd = stat.tile([128, 1], f32, tag="rstd", name="rsf")
            nc.scalar.activation(rstd, mv[:, 1:2], AF.Sqrt, bias=eps_t)
            nc.vector.reciprocal(rstd, rstd)
            xn = hpool.tile([128, T], f32, tag="xn", name="xnf")
            nc.vector.tensor_scalar(
                out=xn,
                in0=X[:, NTT - 1, :],
                scalar1=mv[:, 0:1],
                scalar2=rstd,
                op0=ALU.subtract,
                op1=ALU.mult,
            )
            nc.sync.dma_start(xlast[:, :], xn[127:128, :])

    nc.compile()
    return nc


def _build_phase2():
    import concourse.mybir as mybir
    import concourse.tile as tile
    from concourse import bacc

    f32 = mybir.dt.float32
    bf = mybir.dt.bfloat16
    u16 = mybir.dt.uint16
    AF = mybir.ActivationFunctionType

    nc = bacc.Bacc("TRN2", target_bir_lowering=False)
    xallt = nc.dram_tensor("xallt", [E, NCORES], u16, kind="ExternalInput")
    wtet = nc.dram_tensor("wtet", [E, VSP], u16, kind="ExternalInput")
    lg = nc.dram_tensor("lg", [NCORES, VSP], f32, kind="ExternalOutput")

    with tile.TileContext(nc) as tc:
        with (
            tc.tile_pool(name="s", bufs=1) as s,
            tc.tile_pool(name="w", bufs=6) as w,
            tc.tile_pool(name="o", bufs=4) as o,
            tc.tile_pool(name="p", bufs=4, space="PSUM") as p,
        ):
            xt = s.tile([128, NEO, NCORES], bf)
            nc.sync.dma_start(xt, xallt[:, :].rearrange("(eo p) s -> p eo s", p=128).bitcast(bf))
            for vc in range(VSP // 512):
                pt = p.tile([NCORES, 512], f32, tag="p", name=f"p{vc}")
                for eo in range(NEO):
                    wt = w.tile([128, 512], bf, tag="w", name=f"w{vc}_{eo}")
                    nc.sync.dma_start(
                        wt, wtet[eo * 128 : (eo + 1) * 128, vc * 512 : (vc + 1) * 512].bitcast(bf)
                    )
                    nc.tensor.matmul(pt, xt[:, eo, :], wt, start=(eo == 0), stop=(eo == NEO - 1))
                ot = o.tile([NCORES, 512], f32, tag="o", name=f"o{vc}")
                nc.scalar.activation(ot, pt, AF.Copy)
                nc.sync.dma_start(lg[:, vc * 512 : (vc + 1) * 512], ot)
    nc.compile()
    return nc


def _host_prep(idx, wte, wpe, ln1_w, ln1_b, attn_w, attn_b, proj_w, proj_b,
               ln2_w, ln2_b, fc_w, fc_b, fc2_w, fc2_b, lnf_w, lnf_b, nl):
    import ml_dtypes

    bf = ml_dtypes.bfloat16
    f = np.float32
    idx = np.asarray(idx)
    wte = np.asarray(wte, f)
    wpe = np.asarray(wpe, f)
    x0_all = wte[idx] + wpe[None, :T]  # [8, T, E]

    attn_w = np.asarray(attn_w, f)[:nl]
    ln1_w = np.asarray(ln1_w, f)[:nl]
    fc_w = np.asarray(fc_w, f)[:nl]
    ln2_w = np.asarray(ln2_w, f)[:nl]
    proj_w = np.asarray(proj_w, f)[:nl]
    fc2_w = np.asarray(fc2_w, f)[:nl]

    # fold ln scale into following weights; fold sqrt(1/sqrt(D)) into W_q, W_k
    wqkv = attn_w * ln1_w[:, :, None]
    wqkv[:, :, : 2 * E] *= 1.0 / np.sqrt(np.sqrt(D))
    wfc = fc_w * ln2_w[:, :, None]

    # biases must be zero (true for this model)
    bqkv = np.einsum("le,lec->lc", np.asarray(ln1_b, f)[:nl], attn_w) + np.asarray(attn_b, f)[:nl]
    bfc = np.einsum("le,lec->lc", np.asarray(ln2_b, f)[:nl], fc_w) + np.asarray(fc_b, f)[:nl]
    for nm, b in [("bqkv", bqkv), ("proj_b", np.asarray(proj_b, f)),
                  ("bfc", bfc), ("fc2_b", np.asarray(fc2_b, f)),
                  ("lnf_b", np.asarray(lnf_b, f))]:
        assert np.abs(b).max() == 0.0, f"nonzero bias {nm} not supported by this kernel"

    def b16(w):
        return np.ascontiguousarray(w.astype(bf).view(np.uint16))

    # lhsT-style pack: W [nl, E, C] -> [nl, C/512, NEO, 128, 4, 128]
    def pack_l(w):
        ncol = w.shape[2] // 512
        x = w.reshape(nl, NEO, 128, ncol, 4, 128)  # [l, eo, p, q, ct, m]
        return b16(x.transpose(0, 3, 1, 2, 4, 5))  # [l, q, eo, p, ct, m]

    # rhs-style pack: W [nl, K, N] -> [nl, N/512, K/128, 128, 512]
    def pack_r(w):
        ncol = w.shape[2] // 512
        nk = w.shape[1] // 128
        x = w.reshape(nl, nk, 128, ncol, 512)  # [l, k, p, ch, n]
        return b16(x.transpose(0, 3, 1, 2, 4))  # [l, ch, k, p, n]

    wqk8 = pack_l(wqkv[:, :, : 2 * E])  # [l, 4, NEO, 128, 4, 128]; q=[Qg0,Qg1,Kg0,Kg1]
    wv8 = pack_r(wqkv[:, :, 2 * E :])  # [l, 2, NEO, 128, 512]
    wproj8 = pack_r(proj_w)
    # fc1: [l, 4E] cols -> slabs of 1024, 2 quarter-groups each
    wfcp = pack_l(wfc).reshape(nl, 4, 2, NEO, 128, 4, 128)
    # fc2: contraction 4E as 4 slabs x NEO; cols 1024 as 2x512
    x = fc2_w.reshape(nl, 4, NEO, 128, 2, 512)  # [l, slab, k, p, ch, n]
    wfc28 = b16(x.transpose(0, 1, 4, 2, 3, 5))  # [l, slab, ch, k, p, n]

    wtet = np.ascontiguousarray((wte * np.asarray(lnf_w, f)[None, :]).T)  # [E, V]
    shards = []
    for c in range(NCORES):
        sl = wtet[:, c * VSH : min(V, (c + 1) * VSH)]
        pad = np.zeros((E, VSP), f)
        pad[:, : sl.shape[1]] = sl
        shards.append(np.ascontiguousarray(pad.astype(bf).view(np.uint16)))

    return (
        np.ascontiguousarray(x0_all, f),
        wqk8, wv8, wproj8, wfcp, wfc28,
        shards,
    )


def kernel(idx, wte, wpe, ln1_w, ln1_b, attn_w, attn_b, proj_w, proj_b,
           ln2_w, ln2_b, fc_w, fc_b, fc2_w, fc2_b, lnf_w, lnf_b):
    import ml_dtypes
    from concourse.bass_utils import run_bass_kernel_spmd

    x0_all, wqk8, wv8, wproj8, wfc8, wfc28, shards = _host_prep(
        idx, wte, wpe, ln1_w, ln1_b, attn_w, attn_b, proj_w, proj_b,
        ln2_w, ln2_b, fc_w, fc_b, fc2_w, fc2_b, lnf_w, lnf_b, NL)

    if "p1" not in _CACHE:
        _CACHE["p1"] = _build_phase1(NL)
    nc1 = _CACHE["p1"]
    in_maps = [
        {"x0": x0_all[c], "wqk": wqk8, "wv": wv8, "wproj": wproj8,
         "wfc": wfc8, "wfc2": wfc28}
        for c in range(NCORES)
    ]
    trace = os.environ.get("GPT_TRACE", "0") == "1"
    r1 = run_bass_kernel_spmd(nc1, in_maps, core_ids=list(range(NCORES)), trace=trace)
    _CACHE["r1"] = r1
    xall = np.stack([r1.results[c]["xlast"][0] for c in range(NCORES)])  # [8, E]
    xallt = np.ascontiguousarray(
        xall.T.astype(ml_dtypes.bfloat16).view(np.uint16)
    )  # [E, 8] bf16

    if "p2" not in _CACHE:
        _CACHE["p2"] = _build_phase2()
    nc2 = _CACHE["p2"]
    in_maps2 = [{"xallt": xallt, "wtet": shards[c]} for c in range(NCORES)]
    r2 = run_bass_kernel_spmd(nc2, in_maps2, core_ids=list(range(NCORES)), trace=trace)
    _CACHE["r2"] = r2

    logits = np.zeros((NCORES, 1, V), np.float32)
    for c in range(NCORES):
        w = min(V, (c + 1) * VSH) - c * VSH
        logits[:, 0, c * VSH : c * VSH + w] = r2.results[c]["lg"][:, :w]
    return logits


# revision 19
# speedup vs baseline: 1.1482x; 1.0150x over previous
"""GPT-2 (L=12, E=1024, H=16, T=1024, B=8) forward on 8 Trainium2 NeuronCores.

Data-parallel over batch (1 sequence per core) for the 12 transformer layers;
vocab-parallel lm_head (each core computes a V/8 logits shard for all 8
sequences) as a second tiny NEFF, with the 8 last-position hidden vectors
gathered on host between the phases.

v3 (vs baseline f32r kernel):
  - all matmuls in bf16 (same PE rate as f32r but: half the DMA bytes,
    ldweights at 1 cycle/row, no 4x penalty on <256-wide moving chunks,
    transposes at 1 cycle/row)
  - softmax denominator folded into the AV matmul as a 65th ones-column of
    the V stationary tile (eliminates the separate PE denominator matmul)
  - per-token-tile layernorm pipelined behind proj / fc2-last-slab (PE never
    drains at sublayer boundaries, keeping the HAM clock warm)
  - scores/exp of head h software-pipelined with AV of head h-1
    (double-buffered attT)
  - weights host-packed into per-DMA-tile contiguous layouts (1KB/partition
    lines instead of 256-512B strided)

Host-side preprocessing (all linear folds, no model compute):
  - embedding gather x0 = wte[idx] + wpe  (pure indexing)
  - layernorm scale folded into the following matmul weights
  - sqrt(1/sqrt(D)) folded into both W_q and W_k
  - wte transposed (+ lnf scale) in bf16 for the lm_head
"""

import os
import sys

import numpy as np

sys.path.insert(0, "/opt/trn_rl_repo")

V, BLK, L, H, E = 50257, 1024, 12, 16, 1024
T = 1024
D = E // H  # 64
NCORES = 8
E3 = 3 * E
E4 = 4 * E
NTT = T // 128  # 8 token tiles
NEO = E // 128  # 8 embed tiles
VSH = (V + NCORES - 1) // NCORES  # 6283 vocab shard
VSP = 13 * 512  # 6656 padded shard width
NL = int(os.environ.get("GPT_NL", str(L)))

_CACHE = {}


def _build_phase1(nl):
    import concourse.mybir as mybir
    import concourse.tile as tile
    from concourse import bacc
    from concourse.masks import make_identity

    f32 = mybir.dt.float32
    bf = mybir.dt.bfloat16
    u16 = mybir.dt.uint16
    AF = mybir.ActivationFunctionType
    ALU = mybir.AluOpType

    nc = bacc.Bacc("TRN2", target_bir_lowering=False)

    x0 = nc.dram_tensor("x0", [T, E], f32, kind="ExternalInput")
    # bf16 weights as uint16 carriers (bitcast at DMA), packed per-DMA-tile:
    # lhsT-style [.., eo, p, ct, m]; rhs-style [.., k, p, n]
    wqk = nc.dram_tensor("wqk", [nl, 4, NEO, 128, 4, 128], u16, kind="ExternalInput")
    wv = nc.dram_tensor("wv", [nl, 2, NEO, 128, 512], u16, kind="ExternalInput")
    wproj = nc.dram_tensor("wproj", [nl, 2, NEO, 128, 512], u16, kind="ExternalInput")
    wfc = nc.dram_tensor("wfc", [nl, 4, 2, NEO, 128, 4, 128], u16, kind="ExternalInput")
    wfc2 = nc.dram_tensor("wfc2", [nl, 4, 2, NEO, 128, 512], u16, kind="ExternalInput")
    xlast = nc.dram_tensor("xlast", [1, E], f32, kind="ExternalOutput")

    with tile.TileContext(nc) as tc:
        import contextlib

        ctx = contextlib.ExitStack()
        with ctx:
            singles = ctx.enter_context(tc.tile_pool(name="singles", bufs=1))
            wl = ctx.enter_context(tc.tile_pool(name="wl", bufs=20))  # [128,4,128] bf
            wr = ctx.enter_context(tc.tile_pool(name="wr", bufs=26))  # [128,512] bf
            hpool = ctx.enter_context(tc.tile_pool(name="hpool", bufs=2))
            stat = ctx.enter_context(tc.tile_pool(name="stat", bufs=2))
            bc = ctx.enter_context(tc.tile_pool(name="bc", bufs=2))
            dram = ctx.enter_context(tc.tile_pool(name="dram", bufs=2, space="DRAM"))
            ps = ctx.enter_context(tc.tile_pool(name="ps", bufs=6, space="PSUM"))
            psb = ctx.enter_context(tc.tile_pool(name="psb", bufs=1, space="PSUM"))

            # ---- persistent tiles ----
            X = singles.tile([128, NTT, T], f32)  # residual [t, tt, e]
            HT = singles.tile([128, NEO, T], bf)  # ln-out transposed [e, eo, t]
            AOT = singles.tile([128, NEO, T], bf)  # attn outT [c, co, t]
            scrA = ctx.enter_context(tc.tile_pool(name="scrA", bufs=1))
            scrB = ctx.enter_context(tc.tile_pool(name="scrB", bufs=1))

            identb = singles.tile([128, 128], bf)
            make_identity(nc, identb)
            # maskT[k, q] = 0 if q >= k else -30  (additive, transposed causal)
            maskT = singles.tile([128, 128], f32)
            nc.gpsimd.memset(maskT, 0.0)
            nc.gpsimd.affine_select(
                out=maskT,
                in_=maskT,
                compare_op=ALU.is_ge,
                fill=-30.0,
                base=0,
                pattern=[[1, 128]],
                channel_multiplier=-1,
            )
            eps_t = singles.tile([128, 1], f32)
            nc.gpsimd.memset(eps_t, 1e-5)
            ones_row = singles.tile([1, 1024], f32)
            nc.gpsimd.memset(ones_row, 1.0)

            # ---- load x0 ----
            x0v = x0[:, :].rearrange("(tt p) e -> p tt e", p=128)
            for tt in range(NTT):
                nc.sync.dma_start(X[:, tt, :], x0v[:, tt, :])

            def ln_stats(src, tt):
                """LN(src[t,tt,:]) -> normalized h tile (DVE/ScalarE only)."""
                st = stat.tile([128, 2, 6], f32, tag="bnst", name=f"st{tt}")
                for c in range(2):
                    nc.vector.bn_stats(st[:, c, :], src[:, tt, c * 512 : (c + 1) * 512])
                mv = stat.tile([128, 2], f32, tag="bnmv", name=f"mv{tt}")
                nc.vector.bn_aggr(mv, st)
                rstd = stat.tile([128, 1], f32, tag="rstd", name=f"rs{tt}")
                nc.scalar.activation(rstd, mv[:, 1:2], AF.Sqrt, bias=eps_t)
                nc.vector.reciprocal(rstd, rstd)
                h = hpool.tile([128, T], bf, tag="h", name=f"h{tt}")
                nc.vector.tensor_scalar(
                    out=h,
                    in0=src[:, tt, :],
                    scalar1=mv[:, 0:1],
                    scalar2=rstd,
                    op0=ALU.subtract,
                    op1=ALU.mult,
                )
                return h

            def ln_tr(h, tt):
                """Transpose h into HT[:, :, tt*128:+128] (PE), one tile late."""
                for half in range(2):
                    ptr = ps.tile([128, 4, 128], bf, tag="mm", name=f"ptr{tt}_{half}")
                    for eq in range(4):
                        eo = half * 4 + eq
                        nc.tensor.transpose(ptr[:, eq, :], h[:, eo * 128 : (eo + 1) * 128], identb)
                    nc.scalar.activation(
                        HT[:, half * 4 : half * 4 + 4, tt * 128 : (tt + 1) * 128],
                        ptr,
                        AF.Copy,
                    )

            def mm_lhsw_to_ct(dst, wdram_l, n_ct, act=AF.Copy):
                """dst[:, ct, t] (bf16) = W[ct].T @ HT for ct in range(n_ct).
                wdram_l: [NEO, 128, 4, 128] packed quarter (n_ct=4) or
                list of two for n_ct=8."""
                quarters = wdram_l if isinstance(wdram_l, list) else [wdram_l]
                for qi, wq in enumerate(quarters):
                    wts = []
                    for eo in range(NEO):
                        wt = wl.tile([128, 4, 128], bf, tag="w_l", name=f"wl{qi}_{eo}")
                        nc.sync.dma_start(wt, wq[eo].bitcast(bf))
                        wts.append(wt)
                    for ct in range(4):
                        for ch in range(2):
                            pt = ps.tile([128, 512], f32, tag="mm", name=f"p{qi}_{ct}_{ch}")
                            for eo in range(NEO):
                                nc.tensor.matmul(
                                    pt,
                                    wts[eo][:, ct, :],
                                    HT[:, eo, ch * 512 : (ch + 1) * 512],
                                    start=(eo == 0),
                                    stop=(eo == NEO - 1),
                                )
                            nc.scalar.activation(
                                dst[:, qi * 4 + ct, ch * 512 : (ch + 1) * 512], pt, act
                            )

            def rhs_group(lhsT3, wts, tt, name):
                pt = ps.tile([128, 512], f32, tag="mm", name=name)
                for k in range(NEO):
                    nc.tensor.matmul(
                        pt,
                        lhsT3[:, k, tt * 128 : (tt + 1) * 128],
                        wts[k],
                        start=(k == 0),
                        stop=(k == NEO - 1),
                    )
                return pt

            def load_wr(wdram, n, name):
                wts = []
                for k in range(n):
                    wt = wr.tile([128, 512], bf, tag="w_r", name=f"{name}{k}")
                    nc.sync.dma_start(wt, wdram[k].bitcast(bf))
                    wts.append(wt)
                return wts

            for l in range(nl):
                # ===== attention =====
                if l == 0:
                    hprev = None
                    for tt in range(NTT):
                        hcur = ln_stats(X, tt)
                        if hprev is not None:
                            ln_tr(hprev, tt - 1)
                        hprev = hcur
                    ln_tr(hprev, NTT - 1)
                for g in range(2):  # head groups of 8 heads (512 c-cols)
                    qkg = scrA.tile([128, 8, T], bf, tag="scrA", name=f"qkg{l}_{g}")
                    vatt = scrB.tile([128, 20544], bf, tag="scrB", name=f"vatt{l}_{g}")
                    QG = qkg[:, 0:4, :]
                    KG = qkg[:, 4:8, :]
                    # V with a 65th ones-column per head (softmax denom trick)
                    VG = vatt[:, 0:4160].rearrange("p (j h c) -> p j h c", j=NTT, h=8)
                    # double-buffered non-ragged attT [parity, j, q]
                    attT = vatt[:, 4160:20544].rearrange("p (b j q) -> p b j q", b=2, j=NTT)
                    mm_lhsw_to_ct(QG, wqk[l, g], 4)
                    mm_lhsw_to_ct(KG, wqk[l, 2 + g], 4)

                    vw = load_wr(wv[l, g], NEO, f"vw{g}")
                    for tt in range(NTT):
                        pt = rhs_group(HT, vw, tt, f"pv{tt}")
                        nc.scalar.activation(VG[:, tt, :, 0:64], pt, AF.Copy)
                    nc.gpsimd.memset(VG[:, :, :, 64:65], 1.0)

                    def scores(hh):
                        pb = hh % 2
                        ct, ro = hh // 2, (hh % 2) * 64
                        qT = QG[ro : ro + 64, ct, :]  # [64, 1024] bf16
                        kT = KG[ro : ro + 64, ct, :]
                        for j in range(NTT):
                            q0 = j * 128
                            for ch in range(q0, T, 512):
                                w = min(512, T - ch)
                                pa = ps.tile([128, 512], f32, tag="mm", name=f"pa{j}_{ch}")
                                nc.tensor.matmul(
                                    pa[:, :w],
                                    kT[:, q0 : q0 + 128],
                                    qT[:, ch : ch + w],
                                    start=True,
                                    stop=True,
                                )
                                if ch == q0:  # causal mask on diagonal block
                                    nc.vector.tensor_tensor(pa[:, :128], pa[:, :128], maskT, ALU.add)
                                nc.scalar.activation(attT[:, pb, j, ch : ch + w], pa[:, :w], AF.Exp)

                    def av(hh):
                        pb = hh % 2
                        h_glob = g * 8 + hh
                        av_ps = psb.tile([65, 1024], f32, tag="av", name=f"av{l}_{h_glob}")
                        for j in range(NTT):
                            vsl = VG[:, j, hh, 0:65]
                            for ca in range(2):
                                s = max(ca * 512, j * 128)
                                if s >= (ca + 1) * 512:
                                    continue
                                w = (ca + 1) * 512 - s
                                nc.tensor.matmul(
                                    av_ps[:, s : s + w],
                                    vsl,
                                    attT[:, pb, j, s : s + w],
                                    start=(j == 0),
                                    stop=(j == (3 if ca == 0 else 7)),
                                    skip_group_check=True,
                                )
                        # free av_ps fast: unnormalized AV -> SBUF (bf16)
                        avu = bc.tile([64, 1024], bf, tag="avu", bufs=4, name=f"au{l}_{h_glob}")
                        nc.vector.tensor_scalar(
                            out=avu, in0=av_ps[0:64, :], scalar1=1.0, scalar2=None,
                            op0=ALU.mult,
                        )
                        # denominator: evac row 64 (ScalarE), DMA-spread to
                        # [128, 8] so the DVE reciprocal is 8 els/lane, DMA
                        # back and broadcast-read to 64 partitions
                        dsb = stat.tile([1, 1024], f32, tag="dsb", name=f"ds{l}_{h_glob}")
                        nc.scalar.activation(dsb, av_ps[64:65, :], AF.Copy)
                        dd = dram.tile([1, 1024], f32, tag="dd", name=f"dd{l}_{h_glob}")
                        nc.sync.dma_start(dd, dsb)
                        sp = stat.tile([128, 8], f32, tag="sp", name=f"sp{l}_{h_glob}")
                        nc.sync.dma_start(sp, dd[0, :].rearrange("(p i) -> p i", p=128))
                        nc.vector.reciprocal(sp, sp)
                        spb = stat.tile([128, 8], bf, tag="spb", name=f"sb{l}_{h_glob}")
                        nc.vector.tensor_scalar(
                            out=spb, in0=sp, scalar1=1.0, scalar2=None, op0=ALU.mult
                        )
                        dd2 = dram.tile([1, 1024], bf, tag="dd2", name=f"d2{l}_{h_glob}")
                        nc.sync.dma_start(dd2[0, :].rearrange("(p i) -> p i", p=128), spb)
                        den64 = bc.tile([64, 1024], bf, tag="rdb", name=f"dn{l}_{h_glob}")
                        nc.gpsimd.dma_start(den64, dd2.to_broadcast([64, 1024]))
                        co, ro2 = h_glob // 2, (h_glob % 2) * 64
                        nc.vector.tensor_tensor(
                            AOT[ro2 : ro2 + 64, co, :], avu, den64, ALU.mult
                        )

                    for hh in range(8):
                        scores(hh)
                        if hh > 0:
                            av(hh - 1)
                    av(7)

                # proj + residual, mlp-LN pipelined per token tile
                pw = load_wr(wproj[l, 0], NEO, "pw0") + load_wr(wproj[l, 1], NEO, "pw1")
                hprev = None
                for tt in range(NTT):
                    for ch in range(2):
                        pt = rhs_group(AOT, pw[ch * NEO : (ch + 1) * NEO], tt, f"pp{tt}_{ch}")
                        nc.vector.tensor_tensor(
                            X[:, tt, ch * 512 : (ch + 1) * 512],
                            X[:, tt, ch * 512 : (ch + 1) * 512],
                            pt,
                            ALU.add,
                        )
                    hcur = ln_stats(X, tt)
                    if hprev is not None:
                        ln_tr(hprev, tt - 1)
                    hprev = hcur
                ln_tr(hprev, NTT - 1)

                # ===== mlp =====
                FC2A = scrB.tile([128, NTT, T], f32, tag="scrB", name=f"fc2a{l}")
                for slab in range(4):  # 4E in 4 slabs of 1024
                    H1T = scrA.tile([128, 8, T], bf, tag="scrA", name=f"h1t{l}_{slab}")
                    mm_lhsw_to_ct(
                        H1T, [wfc[l, slab, 0], wfc[l, slab, 1]], 8, act=AF.Gelu_apprx_tanh
                    )
                    f2w = load_wr(wfc2[l, slab, 0], NEO, f"f2a{slab}") + load_wr(
                        wfc2[l, slab, 1], NEO, f"f2b{slab}"
                    )
                    last = slab == 3
                    hprev = None
                    for tt in range(NTT):
                        for ch in range(2):
                            pt = rhs_group(H1T, f2w[ch * NEO : (ch + 1) * NEO], tt, f"pf{slab}_{tt}_{ch}")
                            if slab == 0:
                                nc.vector.tensor_tensor(
                                    FC2A[:, tt, ch * 512 : (ch + 1) * 512],
                                    X[:, tt, ch * 512 : (ch + 1) * 512],
                                    pt,
                                    ALU.add,
                                )
                            elif not last:
                                nc.vector.tensor_tensor(
                                    FC2A[:, tt, ch * 512 : (ch + 1) * 512],
                                    FC2A[:, tt, ch * 512 : (ch + 1) * 512],
                                    pt,
                                    ALU.add,
                                )
                            else:
                                nc.vector.tensor_tensor(
                                    X[:, tt, ch * 512 : (ch + 1) * 512],
                                    FC2A[:, tt, ch * 512 : (ch + 1) * 512],
                                    pt,
                                    ALU.add,
                                )
                        if last and l + 1 < nl:
                            hcur = ln_stats(X, tt)
                            if hprev is not None:
                                ln_tr(hprev, tt - 1)
                            hprev = hcur
                    if last and l + 1 < nl:
                        ln_tr(hprev, NTT - 1)

            # ===== final layernorm on last token tile, emit last row =====
            st = stat.tile([128, 2, 6], f32, tag="bnst", name="stf")
            for c in range(2):
                nc.vector.bn_stats(st[:, c, :], X[:, NTT - 1, c * 512 : (c + 1) * 512])
            mv = stat.tile([128, 2], f32, tag="bnmv", name="mvf")
            nc.vector.bn_aggr(mv, st)
            rstd = stat.tile([128, 1], f32, tag="rstd", name="rsf")
            nc.scalar.activation(rstd, mv[:, 1:2], AF.Sqrt, bias=eps_t)
            nc.vector.reciprocal(rstd, rstd)
            xn = hpool.tile([128, T], f32, tag="xn", name="xnf")
            nc.vector.tensor_scalar(
                out=xn,
                in0=X[:, NTT - 1, :],
                scalar1=mv[:, 0:1],
                scalar2=rstd,
                op0=ALU.subtract,
                op1=ALU.mult,
            )
            nc.sync.dma_start(xlast[:, :], xn[127:128, :])

    nc.compile()
    return nc


def _build_phase2():
    import concourse.mybir as mybir
    import concourse.tile as tile
    from concourse import bacc

    f32 = mybir.dt.float32
    bf = mybir.dt.bfloat16
    u16 = mybir.dt.uint16
    AF = mybir.ActivationFunctionType

    nc = bacc.Bacc("TRN2", target_bir_lowering=False)
    xallt = nc.dram_tensor("xallt", [E, NCORES], u16, kind="ExternalInput")
    wtet = nc.dram_tensor("wtet", [E, VSP], u16, kind="ExternalInput")
    lg = nc.dram_tensor("lg", [NCORES, VSP], f32, kind="ExternalOutput")

    with tile.TileContext(nc) as tc:
        with (
            tc.tile_pool(name="s", bufs=1) as s,
            tc.tile_pool(name="w", bufs=6) as w,
            tc.tile_pool(name="o", bufs=4) as o,
            tc.tile_pool(name="p", bufs=4, space="PSUM") as p,
        ):
            xt = s.tile([128, NEO, NCORES], bf)
            nc.sync.dma_start(xt, xallt[:, :].rearrange("(eo p) s -> p eo s", p=128).bitcast(bf))
            for vc in range(VSP // 512):
                pt = p.tile([NCORES, 512], f32, tag="p", name=f"p{vc}")
                for eo in range(NEO):
                    wt = w.tile([128, 512], bf, tag="w", name=f"w{vc}_{eo}")
                    nc.sync.dma_start(
                        wt, wtet[eo * 128 : (eo + 1) * 128, vc * 512 : (vc + 1) * 512].bitcast(bf)
                    )
                    nc.tensor.matmul(pt, xt[:, eo, :], wt, start=(eo == 0), stop=(eo == NEO - 1))
                ot = o.tile([NCORES, 512], f32, tag="o", name=f"o{vc}")
                nc.scalar.activation(ot, pt, AF.Copy)
                nc.sync.dma_start(lg[:, vc * 512 : (vc + 1) * 512], ot)
    nc.compile()
    return nc


def _host_prep(idx, wte, wpe, ln1_w, ln1_b, attn_w, attn_b, proj_w, proj_b,
               ln2_w, ln2_b, fc_w, fc_b, fc2_w, fc2_b, lnf_w, lnf_b, nl):
    import ml_dtypes

    bf = ml_dtypes.bfloat16
    f = np.float32
    idx = np.asarray(idx)
    wte = np.asarray(wte, f)
    wpe = np.asarray(wpe, f)
    x0_all = wte[idx] + wpe[None, :T]  # [8, T, E]

    attn_w = np.asarray(attn_w, f)[:nl]
    ln1_w = np.asarray(ln1_w, f)[:nl]
    fc_w = np.asarray(fc_w, f)[:nl]
    ln2_w = np.asarray(ln2_w, f)[:nl]
    proj_w = np.asarray(proj_w, f)[:nl]
    fc2_w = np.asarray(fc2_w, f)[:nl]

    # fold ln scale into following weights; fold sqrt(1/sqrt(D)) into W_q, W_k
    wqkv = attn_w * ln1_w[:, :, None]
    wqkv[:, :, : 2 * E] *= 1.0 / np.sqrt(np.sqrt(D))
    wfc = fc_w * ln2_w[:, :, None]

    # biases must be zero (true for this model)
    bqkv = np.einsum("le,lec->lc", np.asarray(ln1_b, f)[:nl], attn_w) + np.asarray(attn_b, f)[:nl]
    bfc = np.einsum("le,lec->lc", np.asarray(ln2_b, f)[:nl], fc_w) + np.asarray(fc_b, f)[:nl]
    for nm, b in [("bqkv", bqkv), ("proj_b", np.asarray(proj_b, f)),
                  ("bfc", bfc), ("fc2_b", np.asarray(fc2_b, f)),
                  ("lnf_b", np.asarray(lnf_b, f))]:
        assert np.abs(b).max() == 0.0, f"nonzero bias {nm} not supported by this kernel"

    def b16(w):
        return np.ascontiguousarray(w.astype(bf).view(np.uint16))

    # lhsT-style pack: W [nl, E, C] -> [nl, C/512, NEO, 128, 4, 128]
    def pack_l(w):
        ncol = w.shape[2] // 512
        x = w.reshape(nl, NEO, 128, ncol, 4, 128)  # [l, eo, p, q, ct, m]
        return b16(x.transpose(0, 3, 1, 2, 4, 5))  # [l, q, eo, p, ct, m]

    # rhs-style pack: W [nl, K, N] -> [nl, N/512, K/128, 128, 512]
    def pack_r(w):
        ncol = w.shape[2] // 512
        nk = w.shape[1] // 128
        x = w.reshape(nl, nk, 128, ncol, 512)  # [l, k, p, ch, n]
        return b16(x.transpose(0, 3, 1, 2, 4))  # [l, ch, k, p, n]

    wqk8 = pack_l(wqkv[:, :, : 2 * E])  # [l, 4, NEO, 128, 4, 128]; q=[Qg0,Qg1,Kg0,Kg1]
    wv8 = pack_r(wqkv[:, :, 2 * E :])  # [l, 2, NEO, 128, 512]
    wproj8 = pack_r(proj_w)
    # fc1: [l, 4E] cols -> slabs of 1024, 2 quarter-groups each
    wfcp = pack_l(wfc).reshape(nl, 4, 2, NEO, 128, 4, 128)
    # fc2: contraction 4E as 4 slabs x NEO; cols 1024 as 2x512
    x = fc2_w.reshape(nl, 4, NEO, 128, 2, 512)  # [l, slab, k, p, ch, n]
    wfc28 = b16(x.transpose(0, 1, 4, 2, 3, 5))  # [l, slab, ch, k, p, n]

    wtet = np.ascontiguousarray((wte * np.asarray(lnf_w, f)[None, :]).T)  # [E, V]
    shards = []
    for c in range(NCORES):
        sl = wtet[:, c * VSH : min(V, (c + 1) * VSH)]
        pad = np.zeros((E, VSP), f)
        pad[:, : sl.shape[1]] = sl
        shards.append(np.ascontiguousarray(pad.astype(bf).view(np.uint16)))

    return (
        np.ascontiguousarray(x0_all, f),
        wqk8, wv8, wproj8, wfcp, wfc28,
        shards,
    )


def kernel(idx, wte, wpe, ln1_w, ln1_b, attn_w, attn_b, proj_w, proj_b,
           ln2_w, ln2_b, fc_w, fc_b, fc2_w, fc2_b, lnf_w, lnf_b):
    import ml_dtypes
    from concourse.bass_utils import run_bass_kernel_spmd

    x0_all, wqk8, wv8, wproj8, wfc8, wfc28, shards = _host_prep(
        idx, wte, wpe, ln1_w, ln1_b, attn_w, attn_b, proj_w, proj_b,
        ln2_w, ln2_b, fc_w, fc_b, fc2_w, fc2_b, lnf_w, lnf_b, NL)

    if "p1" not in _CACHE:
        _CACHE["p1"] = _build_phase1(NL)
    nc1 = _CACHE["p1"]
    in_maps = [
        {"x0": x0_all[c], "wqk": wqk8, "wv": wv8, "wproj": wproj8,
         "wfc": wfc8, "wfc2": wfc28}
        for c in range(NCORES)
    ]
    trace = os.environ.get("GPT_TRACE", "0") == "1"
    r1 = run_bass_kernel_spmd(nc1, in_maps, core_ids=list(range(NCORES)), trace=trace)
    _CACHE["r1"] = r1
    xall = np.stack([r1.results[c]["xlast"][0] for c in range(NCORES)])  # [8, E]
    xallt = np.ascontiguousarray(
        xall.T.astype(ml_dtypes.bfloat16).view(np.uint16)
    )  # [E, 8] bf16

    if "p2" not in _CACHE:
        _CACHE["p2"] = _build_phase2()
    nc2 = _CACHE["p2"]
    in_maps2 = [{"xallt": xallt, "wtet": shards[c]} for c in range(NCORES)]
    r2 = run_bass_kernel_spmd(nc2, in_maps2, core_ids=list(range(NCORES)), trace=trace)
    _CACHE["r2"] = r2

    logits = np.zeros((NCORES, 1, V), np.float32)
    for c in range(NCORES):
        w = min(V, (c + 1) * VSH) - c * VSH
        logits[:, 0, c * VSH : c * VSH + w] = r2.results[c]["lg"][:, :w]
    return logits


# revision 20
# speedup vs baseline: 1.2667x; 1.1032x over previous
"""GPT-2 (L=12, E=1024, H=16, T=1024, B=8) forward on 8 Trainium2 NeuronCores.

Data-parallel over batch (1 sequence per core) for the 12 transformer layers;
vocab-parallel lm_head (each core computes a V/8 logits shard for all 8
sequences) as a second tiny NEFF, with the 8 last-position hidden vectors
gathered on host between the phases.

v3 (vs baseline f32r kernel):
  - all matmuls in bf16 (same PE rate as f32r but: half the DMA bytes,
    ldweights at 1 cycle/row, no 4x penalty on <256-wide moving chunks,
    transposes at 1 cycle/row)
  - softmax denominator folded into the AV matmul as a 65th ones-column of
    the V stationary tile (eliminates the separate PE denominator matmul)
  - per-token-tile layernorm pipelined behind proj / fc2-last-slab (PE never
    drains at sublayer boundaries, keeping the HAM clock warm)
  - scores/exp of head h software-pipelined with AV of head h-1
    (double-buffered attT)
  - weights host-packed into per-DMA-tile contiguous layouts (1KB/partition
    lines instead of 256-512B strided)

Host-side preprocessing (all linear folds, no model compute):
  - embedding gather x0 = wte[idx] + wpe  (pure indexing)
  - layernorm scale folded into the following matmul weights
  - sqrt(1/sqrt(D)) folded into both W_q and W_k
  - wte transposed (+ lnf scale) in bf16 for the lm_head
"""

import os
import sys

import numpy as np

sys.path.insert(0, "/opt/trn_rl_repo")

V, BLK, L, H, E = 50257, 1024, 12, 16, 1024
T = 1024
D = E // H  # 64
NCORES = 8
E3 = 3 * E
E4 = 4 * E
NTT = T // 128  # 8 token tiles
NEO = E // 128  # 8 embed tiles
VSH = (V + NCORES - 1) // NCORES  # 6283 vocab shard
VSP = 13 * 512  # 6656 padded shard width
NL = int(os.environ.get("GPT_NL", str(L)))

_CACHE = {}


def _build_phase1(nl):
    import concourse.mybir as mybir
    import concourse.tile as tile
    from concourse import bacc
    from concourse.masks import make_identity

    f32 = mybir.dt.float32
    bf = mybir.dt.bfloat16
    u16 = mybir.dt.uint16
    AF = mybir.ActivationFunctionType
    ALU = mybir.AluOpType

    nc = bacc.Bacc("TRN2", target_bir_lowering=False)

    x0 = nc.dram_tensor("x0", [T, E], f32, kind="ExternalInput")
    # bf16 weights as uint16 carriers (bitcast at DMA), packed per-DMA-tile:
    # lhsT-style [.., eo, p, ct, m]; rhs-style [.., k, p, n]
    wqk = nc.dram_tensor("wqk", [nl, 4, NEO, 128, 4, 128], u16, kind="ExternalInput")
    wv = nc.dram_tensor("wv", [nl, 2, NEO, 128, 512], u16, kind="ExternalInput")
    wproj = nc.dram_tensor("wproj", [nl, 2, NEO, 128, 512], u16, kind="ExternalInput")
    wfc = nc.dram_tensor("wfc", [nl, 4, 2, NEO, 128, 4, 128], u16, kind="ExternalInput")
    wfc2 = nc.dram_tensor("wfc2", [nl, 4, 2, NEO, 128, 512], u16, kind="ExternalInput")
    xlast = nc.dram_tensor("xlast", [1, E], f32, kind="ExternalOutput")

    with tile.TileContext(nc) as tc:
        import contextlib

        ctx = contextlib.ExitStack()
        with ctx:
            singles = ctx.enter_context(tc.tile_pool(name="singles", bufs=1))
            wl = ctx.enter_context(tc.tile_pool(name="wl", bufs=20))  # [128,4,128] bf
            wr = ctx.enter_context(tc.tile_pool(name="wr", bufs=26))  # [128,512] bf
            hpool = ctx.enter_context(tc.tile_pool(name="hpool", bufs=2))
            stat = ctx.enter_context(tc.tile_pool(name="stat", bufs=2))
            bc = ctx.enter_context(tc.tile_pool(name="bc", bufs=2))
            dram = ctx.enter_context(tc.tile_pool(name="dram", bufs=2, space="DRAM"))
            ps = ctx.enter_context(tc.tile_pool(name="ps", bufs=6, space="PSUM"))
            psb = ctx.enter_context(tc.tile_pool(name="psb", bufs=1, space="PSUM"))

            # ---- persistent tiles ----
            X = singles.tile([128, NTT, T], f32)  # residual [t, tt, e]
            HT = singles.tile([128, NEO, T], bf)  # ln-out transposed [e, eo, t]
            AOT = singles.tile([128, NEO, T], bf)  # attn outT [c, co, t]
            scrA = ctx.enter_context(tc.tile_pool(name="scrA", bufs=1))
            scrB = ctx.enter_context(tc.tile_pool(name="scrB", bufs=1))

            identb = singles.tile([128, 128], bf)
            make_identity(nc, identb)
            # maskT[k, q] = 0 if q >= k else -30  (additive, transposed causal)
            maskT = singles.tile([128, 128], f32)
            nc.gpsimd.memset(maskT, 0.0)
            nc.gpsimd.affine_select(
                out=maskT,
                in_=maskT,
                compare_op=ALU.is_ge,
                fill=-30.0,
                base=0,
                pattern=[[1, 128]],
                channel_multiplier=-1,
            )
            eps_t = singles.tile([128, 1], f32)
            nc.gpsimd.memset(eps_t, 1e-5)
            ones_row = singles.tile([1, 1024], f32)
            nc.gpsimd.memset(ones_row, 1.0)

            # ---- load x0 ----
            x0v = x0[:, :].rearrange("(tt p) e -> p tt e", p=128)
            for tt in range(NTT):
                nc.sync.dma_start(X[:, tt, :], x0v[:, tt, :])

            def ln_stats(src, tt):
                """LN(src[t,tt,:]) -> normalized h tile (DVE/ScalarE only)."""
                st = stat.tile([128, 2, 6], f32, tag="bnst", name=f"st{tt}")
                for c in range(2):
                    nc.vector.bn_stats(st[:, c, :], src[:, tt, c * 512 : (c + 1) * 512])
                mv = stat.tile([128, 2], f32, tag="bnmv", name=f"mv{tt}")
                nc.vector.bn_aggr(mv, st)
                rstd = stat.tile([128, 1], f32, tag="rstd", name=f"rs{tt}")
                nc.scalar.activation(rstd, mv[:, 1:2], AF.Sqrt, bias=eps_t)
                nc.vector.reciprocal(rstd, rstd)
                h = hpool.tile([128, T], bf, tag="h", name=f"h{tt}")
                nc.vector.tensor_scalar(
                    out=h,
                    in0=src[:, tt, :],
                    scalar1=mv[:, 0:1],
                    scalar2=rstd,
                    op0=ALU.subtract,
                    op1=ALU.mult,
                )
                return h

            def ln_tr(h, tt):
                """Transpose h into HT[:, :, tt*128:+128] (PE), one tile late."""
                for half in range(2):
                    ptr = ps.tile([128, 4, 128], bf, tag="mm", name=f"ptr{tt}_{half}")
                    for eq in range(4):
                        eo = half * 4 + eq
                        nc.tensor.transpose(ptr[:, eq, :], h[:, eo * 128 : (eo + 1) * 128], identb)
                    nc.scalar.activation(
                        HT[:, half * 4 : half * 4 + 4, tt * 128 : (tt + 1) * 128],
                        ptr,
                        AF.Copy,
                    )

            def mm_lhsw_to_ct(dst, wdram_l, n_ct, act=AF.Copy):
                """dst[:, ct, t] (bf16) = W[ct].T @ HT for ct in range(n_ct).
                wdram_l: [NEO, 128, 4, 128] packed quarter (n_ct=4) or
                list of two for n_ct=8."""
                quarters = wdram_l if isinstance(wdram_l, list) else [wdram_l]
                for qi, wq in enumerate(quarters):
                    wts = []
                    for eo in range(NEO):
                        wt = wl.tile([128, 4, 128], bf, tag="w_l", name=f"wl{qi}_{eo}")
                        nc.sync.dma_start(wt, wq[eo].bitcast(bf))
                        wts.append(wt)
                    for ct in range(4):
                        for ch in range(2):
                            pt = ps.tile([128, 512], f32, tag="mm", name=f"p{qi}_{ct}_{ch}")
                            for eo in range(NEO):
                                nc.tensor.matmul(
                                    pt,
                                    wts[eo][:, ct, :],
                                    HT[:, eo, ch * 512 : (ch + 1) * 512],
                                    start=(eo == 0),
                                    stop=(eo == NEO - 1),
                                )
                            nc.scalar.activation(
                                dst[:, qi * 4 + ct, ch * 512 : (ch + 1) * 512], pt, act
                            )

            def rhs_group(lhsT3, wts, tt, name):
                pt = ps.tile([128, 512], f32, tag="mm", name=name)
                for k in range(NEO):
                    nc.tensor.matmul(
                        pt,
                        lhsT3[:, k, tt * 128 : (tt + 1) * 128],
                        wts[k],
                        start=(k == 0),
                        stop=(k == NEO - 1),
                    )
                return pt

            def load_wr(wdram, n, name):
                wts = []
                for k in range(n):
                    wt = wr.tile([128, 512], bf, tag="w_r", name=f"{name}{k}")
                    nc.sync.dma_start(wt, wdram[k].bitcast(bf))
                    wts.append(wt)
                return wts

            for l in range(nl):
                # ===== attention =====
                if l == 0:
                    hprev = None
                    for tt in range(NTT):
                        hcur = ln_stats(X, tt)
                        if hprev is not None:
                            ln_tr(hprev, tt - 1)
                        hprev = hcur
                    ln_tr(hprev, NTT - 1)
                for g in range(2):  # head groups of 8 heads (512 c-cols)
                    qkg = scrA.tile([128, 8, T], bf, tag="scrA", name=f"qkg{l}_{g}")
                    vatt = scrB.tile([128, 20544], bf, tag="scrB", name=f"vatt{l}_{g}")
                    QG = qkg[:, 0:4, :]
                    KG = qkg[:, 4:8, :]
                    # V with a 65th ones-column per head (softmax denom trick)
                    VG = vatt[:, 0:4160].rearrange("p (j h c) -> p j h c", j=NTT, h=8)
                    # double-buffered non-ragged attT [parity, j, q]
                    attT = vatt[:, 4160:20544].rearrange("p (b j q) -> p b j q", b=2, j=NTT)
                    mm_lhsw_to_ct(QG, wqk[l, g], 4)
                    mm_lhsw_to_ct(KG, wqk[l, 2 + g], 4)

                    vw = load_wr(wv[l, g], NEO, f"vw{g}")
                    for tt in range(NTT):
                        pt = rhs_group(HT, vw, tt, f"pv{tt}")
                        nc.scalar.activation(VG[:, tt, :, 0:64], pt, AF.Copy)
                    nc.gpsimd.memset(VG[:, :, :, 64:65], 1.0)

                    def scores(hh):
                        pb = hh % 2
                        ct, ro = hh // 2, (hh % 2) * 64
                        qT = QG[ro : ro + 64, ct, :]  # [64, 1024] bf16
                        kT = KG[ro : ro + 64, ct, :]
                        for j in range(NTT):
                            q0 = j * 128
                            for ch in range(q0, T, 512):
                                w = min(512, T - ch)
                                pa = ps.tile([128, 512], f32, tag="mm", name=f"pa{j}_{ch}")
                                nc.tensor.matmul(
                                    pa[:, :w],
                                    kT[:, q0 : q0 + 128],
                                    qT[:, ch : ch + w],
                                    start=True,
                                    stop=True,
                                )
                                if ch == q0:  # causal mask on diagonal block
                                    nc.vector.tensor_tensor(pa[:, :128], pa[:, :128], maskT, ALU.add)
                                nc.scalar.activation(attT[:, pb, j, ch : ch + w], pa[:, :w], AF.Exp)

                    def av(hh):
                        pb = hh % 2
                        h_glob = g * 8 + hh
                        av_ps = psb.tile([65, 1024], f32, tag="av", name=f"av{l}_{h_glob}")
                        for j in range(NTT):
                            vsl = VG[:, j, hh, 0:65]
                            for ca in range(2):
                                s = max(ca * 512, j * 128)
                                if s >= (ca + 1) * 512:
                                    continue
                                w = (ca + 1) * 512 - s
                                nc.tensor.matmul(
                                    av_ps[:, s : s + w],
                                    vsl,
                                    attT[:, pb, j, s : s + w],
                                    start=(j == 0),
                                    stop=(j == (3 if ca == 0 else 7)),
                                    skip_group_check=True,
                                )
                        # free av_ps fast: unnormalized AV -> SBUF (bf16)
                        avu = bc.tile([64, 1024], bf, tag="avu", bufs=4, name=f"au{l}_{h_glob}")
                        nc.vector.tensor_scalar(
                            out=avu, in0=av_ps[0:64, :], scalar1=1.0, scalar2=None,
                            op0=ALU.mult,
                        )
                        den64 = bc.tile([64, 1024], bf, tag="rdb", name=f"dn{l}_{h_glob}")
                        if g == 1 and hh == 7:
                            # last head: low-latency path (proj waits on this)
                            rf = stat.tile([1, 1024], f32, tag="dsb", name=f"rf{l}")
                            nc.vector.reciprocal(rf, av_ps[64:65, :])
                            rb = stat.tile([1, 1024], bf, tag="rfb", name=f"rb{l}")
                            nc.vector.tensor_scalar(
                                out=rb, in0=rf, scalar1=1.0, scalar2=None, op0=ALU.mult
                            )
                            nc.gpsimd.partition_broadcast(den64, rb)
                        else:
                            # denominator: evac row 64 (ScalarE), DMA-spread to
                            # [128, 8] so the DVE reciprocal is 8 els/lane, DMA
                            # back and broadcast-read to 64 partitions
                            dsb = stat.tile([1, 1024], f32, tag="dsb", name=f"ds{l}_{h_glob}")
                            nc.scalar.activation(dsb, av_ps[64:65, :], AF.Copy)
                            dd = dram.tile([1, 1024], f32, tag="dd", name=f"dd{l}_{h_glob}")
                            nc.sync.dma_start(dd, dsb)
                            sp = stat.tile([128, 8], f32, tag="sp", name=f"sp{l}_{h_glob}")
                            nc.sync.dma_start(sp, dd[0, :].rearrange("(p i) -> p i", p=128))
                            nc.vector.reciprocal(sp, sp)
                            spb = stat.tile([128, 8], bf, tag="spb", name=f"sb{l}_{h_glob}")
                            nc.vector.tensor_scalar(
                                out=spb, in0=sp, scalar1=1.0, scalar2=None, op0=ALU.mult
                            )
                            dd2 = dram.tile([1, 1024], bf, tag="dd2", name=f"d2{l}_{h_glob}")
                            nc.sync.dma_start(dd2[0, :].rearrange("(p i) -> p i", p=128), spb)
                            nc.gpsimd.dma_start(den64, dd2.to_broadcast([64, 1024]))
                        co, ro2 = h_glob // 2, (h_glob % 2) * 64
                        nc.vector.tensor_tensor(
                            AOT[ro2 : ro2 + 64, co, :], avu, den64, ALU.mult
                        )

                    for hh in range(8):
                        scores(hh)
                        if hh > 0:
                            av(hh - 1)
                    av(7)

                # proj + residual, mlp-LN pipelined per token tile
                pw = load_wr(wproj[l, 0], NEO, "pw0") + load_wr(wproj[l, 1], NEO, "pw1")
                hprev = None
                for tt in range(NTT):
                    for ch in range(2):
                        pt = rhs_group(AOT, pw[ch * NEO : (ch + 1) * NEO], tt, f"pp{tt}_{ch}")
                        nc.vector.tensor_tensor(
                            X[:, tt, ch * 512 : (ch + 1) * 512],
                            X[:, tt, ch * 512 : (ch + 1) * 512],
                            pt,
                            ALU.add,
                        )
                    hcur = ln_stats(X, tt)
                    if hprev is not None:
                        ln_tr(hprev, tt - 1)
                    hprev = hcur
                ln_tr(hprev, NTT - 1)

                # ===== mlp =====
                FC2A = scrB.tile([128, NTT, T], f32, tag="scrB", name=f"fc2a{l}")
                for slab in range(4):  # 4E in 4 slabs of 1024
                    H1T = scrA.tile([128, 8, T], bf, tag="scrA", name=f"h1t{l}_{slab}")
                    mm_lhsw_to_ct(
                        H1T, [wfc[l, slab, 0], wfc[l, slab, 1]], 8, act=AF.Gelu_apprx_tanh
                    )
                    f2w = load_wr(wfc2[l, slab, 0], NEO, f"f2a{slab}") + load_wr(
                        wfc2[l, slab, 1], NEO, f"f2b{slab}"
                    )
                    last = slab == 3
                    hprev = None
                    for tt in range(NTT):
                        for ch in range(2):
                            pt = rhs_group(H1T, f2w[ch * NEO : (ch + 1) * NEO], tt, f"pf{slab}_{tt}_{ch}")
                            if slab == 0:
                                nc.vector.tensor_tensor(
                                    FC2A[:, tt, ch * 512 : (ch + 1) * 512],
                                    X[:, tt, ch * 512 : (ch + 1) * 512],
                                    pt,
                                    ALU.add,
                                )
                            elif not last:
                                nc.vector.tensor_tensor(
                                    FC2A[:, tt, ch * 512 : (ch + 1) * 512],
                                    FC2A[:, tt, ch * 512 : (ch + 1) * 512],
                                    pt,
                                    ALU.add,
                                )
                            else:
                                nc.vector.tensor_tensor(
                                    X[:, tt, ch * 512 : (ch + 1) * 512],
                                    FC2A[:, tt, ch * 512 : (ch + 1) * 512],
                                    pt,
                                    ALU.add,
                                )
                        if last and l + 1 < nl:
                            hcur = ln_stats(X, tt)
                            if hprev is not None:
                                ln_tr(hprev, tt - 1)
                            hprev = hcur
                    if last and l + 1 < nl:
                        ln_tr(hprev, NTT - 1)

            # ===== final layernorm on last token tile, emit last row =====
            st = stat.tile([128, 2, 6], f32, tag="bnst", name="stf")
            for c in range(2):
                nc.vector.bn_stats(st[:, c, :], X[:, NTT - 1, c * 512 : (c + 1) * 512])
            mv = stat.tile([128, 2], f32, tag="bnmv", name="mvf")
            nc.vector.bn_aggr(mv, st)
            rstd = stat.tile([128, 1], f32, tag="rstd", name="rsf")
            nc.scalar.activation(rstd, mv[:, 1:2], AF.Sqrt, bias=eps_t)
            nc.vector.reciprocal(rstd, rstd)
            xn = hpool.tile([128, T], f32, tag="xn", name="xnf")
            nc.vector.tensor_scalar(
                out=xn,
                in0=X[:, NTT - 1, :],
                scalar1=mv[:, 0:1],
                scalar2=rstd,
                op0=ALU.subtract,
                op1=ALU.mult,
            )
            nc.sync.dma_start(xlast[:, :], xn[127:128, :])

    nc.compile()
    return nc


def _build_phase2():
    import concourse.mybir as mybir
    import concourse.tile as tile
    from concourse import bacc

    f32 = mybir.dt.float32
    bf = mybir.dt.bfloat16
    u16 = mybir.dt.uint16
    AF = mybir.ActivationFunctionType

    nc = bacc.Bacc("TRN2", target_bir_lowering=False)
    xallt = nc.dram_tensor("xallt", [E, NCORES], u16, kind="ExternalInput")
    wtet = nc.dram_tensor("wtet", [E, VSP], u16, kind="ExternalInput")
    lg = nc.dram_tensor("lg", [NCORES, VSP], f32, kind="ExternalOutput")

    with tile.TileContext(nc) as tc:
        with (
            tc.tile_pool(name="s", bufs=1) as s,
            tc.tile_pool(name="w", bufs=14) as w,
            tc.tile_pool(name="o", bufs=4) as o,
            tc.tile_pool(name="p", bufs=4, space="PSUM") as p,
        ):
            xt = s.tile([128, NEO, NCORES], bf)
            nc.sync.dma_start(xt, xallt[:, :].rearrange("(eo p) s -> p eo s", p=128).bitcast(bf))
            for vc in range(VSP // 512):
                pt = p.tile([NCORES, 512], f32, tag="p", name=f"p{vc}")
                for eo in range(NEO):
                    wt = w.tile([128, 512], bf, tag="w", name=f"w{vc}_{eo}")
                    nc.sync.dma_start(
                        wt, wtet[eo * 128 : (eo + 1) * 128, vc * 512 : (vc + 1) * 512].bitcast(bf)
                    )
                    nc.tensor.matmul(pt, xt[:, eo, :], wt, start=(eo == 0), stop=(eo == NEO - 1))
                ot = o.tile([NCORES, 512], f32, tag="o", name=f"o{vc}")
                nc.scalar.activation(ot, pt, AF.Copy)
                nc.sync.dma_start(lg[:, vc * 512 : (vc + 1) * 512], ot)
    nc.compile()
    return nc


def _host_prep(idx, wte, wpe, ln1_w, ln1_b, attn_w, attn_b, proj_w, proj_b,
               ln2_w, ln2_b, fc_w, fc_b, fc2_w, fc2_b, lnf_w, lnf_b, nl):
    import ml_dtypes

    bf = ml_dtypes.bfloat16
    f = np.float32
    idx = np.asarray(idx)
    wte = np.asarray(wte, f)
    wpe = np.asarray(wpe, f)
    x0_all = wte[idx] + wpe[None, :T]  # [8, T, E]

    attn_w = np.asarray(attn_w, f)[:nl]
    ln1_w = np.asarray(ln1_w, f)[:nl]
    fc_w = np.asarray(fc_w, f)[:nl]
    ln2_w = np.asarray(ln2_w, f)[:nl]
    proj_w = np.asarray(proj_w, f)[:nl]
    fc2_w = np.asarray(fc2_w, f)[:nl]

    # fold ln scale into following weights; fold sqrt(1/sqrt(D)) into W_q, W_k
    wqkv = attn_w * ln1_w[:, :, None]
    wqkv[:, :, : 2 * E] *= 1.0 / np.sqrt(np.sqrt(D))
    wfc = fc_w * ln2_w[:, :, None]

    # biases must be zero (true for this model)
    bqkv = np.einsum("le,lec->lc", np.asarray(ln1_b, f)[:nl], attn_w) + np.asarray(attn_b, f)[:nl]
    bfc = np.einsum("le,lec->lc", np.asarray(ln2_b, f)[:nl], fc_w) + np.asarray(fc_b, f)[:nl]
    for nm, b in [("bqkv", bqkv), ("proj_b", np.asarray(proj_b, f)),
                  ("bfc", bfc), ("fc2_b", np.asarray(fc2_b, f)),
                  ("lnf_b", np.asarray(lnf_b, f))]:
        assert np.abs(b).max() == 0.0, f"nonzero bias {nm} not supported by this kernel"

    def b16(w):
        return np.ascontiguousarray(w.astype(bf).view(np.uint16))

    # lhsT-style pack: W [nl, E, C] -> [nl, C/512, NEO, 128, 4, 128]
    def pack_l(w):
        ncol = w.shape[2] // 512
        x = w.reshape(nl, NEO, 128, ncol, 4, 128)  # [l, eo, p, q, ct, m]
        return b16(x.transpose(0, 3, 1, 2, 4, 5))  # [l, q, eo, p, ct, m]

    # rhs-style pack: W [nl, K, N] -> [nl, N/512, K/128, 128, 512]
    def pack_r(w):
        ncol = w.shape[2] // 512
        nk = w.shape[1] // 128
        x = w.reshape(nl, nk, 128, ncol, 512)  # [l, k, p, ch, n]
        return b16(x.transpose(0, 3, 1, 2, 4))  # [l, ch, k, p, n]

    wqk8 = pack_l(wqkv[:, :, : 2 * E])  # [l, 4, NEO, 128, 4, 128]; q=[Qg0,Qg1,Kg0,Kg1]
    wv8 = pack_r(wqkv[:, :, 2 * E :])  # [l, 2, NEO, 128, 512]
    wproj8 = pack_r(proj_w)
    # fc1: [l, 4E] cols -> slabs of 1024, 2 quarter-groups each
    wfcp = pack_l(wfc).reshape(nl, 4, 2, NEO, 128, 4, 128)
    # fc2: contraction 4E as 4 slabs x NEO; cols 1024 as 2x512
    x = fc2_w.reshape(nl, 4, NEO, 128, 2, 512)  # [l, slab, k, p, ch, n]
    wfc28 = b16(x.transpose(0, 1, 4, 2, 3, 5))  # [l, slab, ch, k, p, n]

    wtet = np.ascontiguousarray((wte * np.asarray(lnf_w, f)[None, :]).T)  # [E, V]
    shards = []
    for c in range(NCORES):
        sl = wtet[:, c * VSH : min(V, (c + 1) * VSH)]
        pad = np.zeros((E, VSP), f)
        pad[:, : sl.shape[1]] = sl
        shards.append(np.ascontiguousarray(pad.astype(bf).view(np.uint16)))

    return (
        np.ascontiguousarray(x0_all, f),
        wqk8, wv8, wproj8, wfcp, wfc28,
        shards,
    )


def kernel(idx, wte, wpe, ln1_w, ln1_b, attn_w, attn_b, proj_w, proj_b,
           ln2_w, ln2_b, fc_w, fc_b, fc2_w, fc2_b, lnf_w, lnf_b):
    import ml_dtypes
    from concourse.bass_utils import run_bass_kernel_spmd

    x0_all, wqk8, wv8, wproj8, wfc8, wfc28, shards = _host_prep(
        idx, wte, wpe, ln1_w, ln1_b, attn_w, attn_b, proj_w, proj_b,
        ln2_w, ln2_b, fc_w, fc_b, fc2_w, fc2_b, lnf_w, lnf_b, NL)

    if "p1" not in _CACHE:
        _CACHE["p1"] = _build_phase1(NL)
    nc1 = _CACHE["p1"]
    in_maps = [
        {"x0": x0_all[c], "wqk": wqk8, "wv": wv8, "wproj": wproj8,
         "wfc": wfc8, "wfc2": wfc28}
        for c in range(NCORES)
    ]
    trace = os.environ.get("GPT_TRACE", "0") == "1"
    r1 = run_bass_kernel_spmd(nc1, in_maps, core_ids=list(range(NCORES)), trace=trace)
    _CACHE["r1"] = r1
    xall = np.stack([r1.results[c]["xlast"][0] for c in range(NCORES)])  # [8, E]
    xallt = np.ascontiguousarray(
        xall.T.astype(ml_dtypes.bfloat16).view(np.uint16)
    )  # [E, 8] bf16

    if "p2" not in _CACHE:
        _CACHE["p2"] = _build_phase2()
    nc2 = _CACHE["p2"]
    in_maps2 = [{"xallt": xallt, "wtet": shards[c]} for c in range(NCORES)]
    r2 = run_bass_kernel_spmd(nc2, in_maps2, core_ids=list(range(NCORES)), trace=trace)
    _CACHE["r2"] = r2

    logits = np.zeros((NCORES, 1, V), np.float32)
    for c in range(NCORES):
        w = min(V, (c + 1) * VSH) - c * VSH
        logits[:, 0, c * VSH : c * VSH + w] = r2.results[c]["lg"][:, :w]
    return logits
